# revision 1
# baseline (speedup 1.0000x reference)
"""Trainium2 Bass kernel for nn_CrossAttention (B=4, N=4096, T=256, DIM=1024,
16 heads x 64 dim, cosine-sim attention with null-kv token, LN in/ctx/out).

Sharding: data-parallel over query rows. Core c handles batch b=c//2, query
rows (c%2)*2048 : (c%2)*2048+2048. The kv projections (tiny: T=256) are
computed redundantly per core; no collectives are needed. Each core returns
its [2048, 1024] output slice; the host reassembles the full [4,4096,1024].

All matmuls run with fp16 inputs (fp32 PSUM accumulation) at 1 PE cycle/row;
fp32/f32r matmuls crash the exec unit on this runtime and bf16 would cost 8x
mantissa. The attention operands are l2-normalized so fp16 conditioning is
ideal. Statistics (LN mean/var, l2 norms, softmax sums, reciprocals) are all
computed in fp32 from fp32 PSUM values.

Algebraic folds that remove elementwise work:
  - ln_in gain/bias are folded into Wq on device (Wq' = diag(g) Wq, plus a
    rank-1 bias matmul b @ Wq injected into each q accumulation group), so
    the x LayerNorm emits only the (x - m) * rstd activation pass.
  - q_scale * k_scale is folded into the k side only (scores contract
    q_hat . k_hat * (qs*ks) per dim).
  - softmax needs no max-subtraction (scores bounded by SCALE) and no
    partition reduction: exp with fused scale=8, bias=ln(1/256) (cancels in
    the division, keeps fp16 exps in range); kv-sums ride a ones-column of
    v' (row 64 of the outT psum); 1/sum is partition-broadcast by a K=1
    ones matmul + DVE reciprocal.
"""

import numpy as np
from contextlib import ExitStack

import concourse.bass as bass
import concourse.tile as tile
from concourse import bacc, mybir
from concourse.bass_utils import run_bass_kernel_spmd
from concourse.masks import make_identity

F32 = mybir.dt.float32
F16 = mybir.dt.float16
AF = mybir.ActivationFunctionType
AX = mybir.AxisListType

DIM = 1024
HEADS = 16
HD = 64
T = 256
TK = T + 1
SCALE = 8.0
EXPB = -5.545177444479562  # ln(1/256)
LN_EPS = 1e-5
NORM_EPS = 1e-12
N_CORES = 8
ROWS = 2048
QMACRO = 512
NSUB = QMACRO // 128
NMACRO = ROWS // QMACRO


def _emit_ln_stats(nc, pool_small, in_aps, eps_tile):
    """in_aps: list of [128, 512] APs covering the 1024 row. Returns
    (rstd [128,1], negmr [128,1]) fp32 tiles for (x - m) * rstd."""
    stats = pool_small.tile([128, 2, 6], F32, tag="lnstats")
    for i, ap in enumerate(in_aps):
        nc.vector.bn_stats(out=stats[:, i, :], in_=ap)
    mv = pool_small.tile([128, 2], F32, tag="lnmv")
    nc.vector.bn_aggr(out=mv[:], in_=stats[:])
    std = pool_small.tile([128, 1], F32, tag="lnstd")
    nc.scalar.activation(out=std[:], in_=mv[:, 1:2], func=AF.Sqrt,
                         bias=eps_tile[:], scale=1.0)
    rstd = pool_small.tile([128, 1], F32, tag="lnrstd")
    nc.vector.reciprocal(rstd[:], std[:])
    negmr = pool_small.tile([128, 1], F32, tag="lnnegmr")
    nc.vector.scalar_tensor_tensor(out=negmr[:], in0=mv[:, 0:1], scalar=-1.0,
                                   in1=rstd[:], op0=mybir.AluOpType.mult,
                                   op1=mybir.AluOpType.mult)
    return rstd, negmr


def _emit_l2norm_heads(nc, pool_small, sq_pool, out_ap3, in_half_aps, scale_tile):
    """in_half_aps: [pa, pb] fp32 psum APs [128, 512] (heads 0-7, 8-15).
    out_ap3: [128, 16, 64] fp16 sbuf AP. out = in/max(||in_head||,eps)
    (* scale_tile [128,64] if given)."""
    sq = sq_pool.tile([128, 1024], F32, tag="sq")
    nc.scalar.activation(out=sq[:, 0:512], in_=in_half_aps[0], func=AF.Square,
                         bias=0.0, scale=1.0)
    nc.scalar.activation(out=sq[:, 512:1024], in_=in_half_aps[1], func=AF.Square,
                         bias=0.0, scale=1.0)
    ssq = pool_small.tile([128, 16], F32, tag="ssq")
    nc.vector.reduce_sum(out=ssq[:], in_=sq[:].rearrange("p (h d) -> p h d", d=HD),
                         axis=AX.X)
    norm = pool_small.tile([128, 16], F32, tag="l2norm")
    nc.scalar.activation(out=norm[:], in_=ssq[:], func=AF.Sqrt, bias=0.0, scale=1.0)
    nc.vector.tensor_scalar_max(norm[:], norm[:], NORM_EPS)
    rn = pool_small.tile([128, 16], F32, tag="l2rn")
    nc.vector.reciprocal(rn[:], norm[:])
    for i in range(2):
        h0 = i * 8
        out_h = out_ap3[:, h0:h0 + 8, :]
        in3 = in_half_aps[i].rearrange("p (h d) -> p h d", d=HD)
        nc.vector.tensor_mul(
            out_h, in3,
            rn[:, h0:h0 + 8].unsqueeze(-1).broadcast_to([128, 8, HD]))
        if scale_tile is not None:
            nc.vector.tensor_mul(
                out_h, out_h,
                scale_tile[:].unsqueeze(1).broadcast_to([128, 8, HD]))


def _load_bcast(nc, dst_tile, dram_ap, parts=128):
    ap = bass.AP(tensor=dram_ap.tensor, offset=dram_ap.offset,
                 ap=[[0, parts]] + dram_ap.ap)
    nc.sync.dma_start(out=dst_tile[:parts, :], in_=ap)


def _load_cols(nc, dst_tile, dram_ap):
    """Load a [1024] dram vector as [128, 8] (dst[p, kc] = v[kc*128+p])."""
    ap = bass.AP(tensor=dram_ap.tensor, offset=dram_ap.offset,
                 ap=[[1, 128], [128, 8]])
    nc.sync.dma_start(out=dst_tile[:, :], in_=ap)


def build_nc():
    nc = bacc.Bacc("TRN2", debug=False)

    XS = nc.dram_tensor("xs", [ROWS, DIM], F32, kind="ExternalInput")
    CTX = nc.dram_tensor("ctx", [T, DIM], F32, kind="ExternalInput")
    WQ = nc.dram_tensor("Wq", [DIM, DIM], F16, kind="ExternalInput")
    WKV = nc.dram_tensor("Wkv", [DIM, 2 * DIM], F16, kind="ExternalInput")
    WO = nc.dram_tensor("Wo", [DIM, DIM], F16, kind="ExternalInput")
    NKV = nc.dram_tensor("null_kv", [2, HD], F32, kind="ExternalInput")
    QS = nc.dram_tensor("q_scale", [HD], F32, kind="ExternalInput")
    KS = nc.dram_tensor("k_scale", [HD], F32, kind="ExternalInput")
    LIG = nc.dram_tensor("ln_in_g", [DIM], F32, kind="ExternalInput")
    LIB = nc.dram_tensor("ln_in_b", [DIM], F32, kind="ExternalInput")
    LCG = nc.dram_tensor("ln_ctx_g", [DIM], F32, kind="ExternalInput")
    LCB = nc.dram_tensor("ln_ctx_b", [DIM], F32, kind="ExternalInput")
    LOG = nc.dram_tensor("ln_out_g", [DIM], F32, kind="ExternalInput")
    LOB = nc.dram_tensor("ln_out_b", [DIM], F32, kind="ExternalInput")
    OUT = nc.dram_tensor("out", [ROWS, DIM], F32, kind="ExternalOutput")

    with tile.TileContext(nc) as tc, ExitStack() as ctx:
        consts = ctx.enter_context(tc.tile_pool(name="consts", bufs=1))
        weights = ctx.enter_context(tc.tile_pool(name="weights", bufs=1))
        kvpool = ctx.enter_context(tc.tile_pool(name="kvpool", bufs=1))
        small = ctx.enter_context(tc.tile_pool(name="small", bufs=2))
        sq_pool = ctx.enter_context(tc.tile_pool(name="sqp", bufs=1))

        # PSUM: mi(2) + half(2) + sc(2) + o(2) = 8 banks
        ps_mi = ctx.enter_context(tc.tile_pool(name="ps_mi", bufs=2, space="PSUM"))
        ps_half = ctx.enter_context(tc.tile_pool(name="ps_half", bufs=2, space="PSUM"))
        ps_sc = ctx.enter_context(tc.tile_pool(name="ps_sc", bufs=1, space="PSUM"))
        ps_o = ctx.enter_context(tc.tile_pool(name="ps_o", bufs=2, space="PSUM"))

        ident = consts.tile([128, 128], F16)
        make_identity(nc, ident)
        eps_tile = consts.tile([128, 1], F32)
        nc.vector.memset(eps_tile[:], LN_EPS)
        onesf = consts.tile([128, 1], F32)
        nc.vector.memset(onesf[:], 1.0)
        expb = consts.tile([128, 1], F32)
        nc.vector.memset(expb[:], EXPB)
        ones_t = consts.tile([128, HD], F16)
        nc.vector.tensor_copy(ones_t[:], onesf[:, 0:1].broadcast_to([128, HD]))
        ones_row = consts.tile([1, 128], F16)
        nc.vector.tensor_copy(ones_row[0:1, :], onesf[0:1, 0:1].broadcast_to([1, 128]))

        log_ = consts.tile([128, DIM], F32)
        lob = consts.tile([128, DIM], F32)
        _load_bcast(nc, log_, LOG[:])
        _load_bcast(nc, lob, LOB[:])
        lcg = consts.tile([128, DIM], F32)
        lcb = consts.tile([128, DIM], F32)
        _load_bcast(nc, lcg, LCG[:])
        _load_bcast(nc, lcb, LCB[:])
        qsc = consts.tile([128, HD], F32)
        _load_bcast(nc, qsc, QS[:])
        ksc = consts.tile([128, HD], F32)
        _load_bcast(nc, ksc, KS[:])
        kqsc = consts.tile([128, HD], F32)
        nc.vector.tensor_mul(kqsc[:], ksc[:], qsc[:])
        gcol = consts.tile([128, 8], F32)
        _load_cols(nc, gcol, LIG[:])
        bcol = consts.tile([128, 8], F32)
        _load_cols(nc, bcol, LIB[:])
        bcol16 = consts.tile([128, 8], F16)
        nc.vector.tensor_copy(bcol16[:], bcol[:])

        wq_sb = weights.tile([128, 8, DIM], F16)
        for kc in range(8):
            nc.sync.dma_start(out=wq_sb[:, kc, :], in_=WQ[kc * 128:(kc + 1) * 128, :])
        wo_sb = weights.tile([128, 8, DIM], F16)
        for kc in range(8):
            nc.sync.dma_start(out=wo_sb[:, kc, :], in_=WO[kc * 128:(kc + 1) * 128, :])

        # qbias = ln_in_b @ Wq (rank-1), computed before Wq is gain-scaled
        qbias = kvpool.tile([1, DIM], F16)
        for half in range(2):
            pqb = ps_mi.tile([1, 512], F32, tag="mi")
            for kc in range(8):
                nc.tensor.matmul(pqb[:], lhsT=bcol16[:, kc:kc + 1],
                                 rhs=wq_sb[:, kc, half * 512:(half + 1) * 512],
                                 start=(kc == 0), stop=(kc == 7))
            nc.vector.tensor_copy(qbias[0:1, half * 512:(half + 1) * 512], pqb[:])
        # Wq' = diag(ln_in_g) @ Wq
        for kc in range(8):
            nc.vector.tensor_scalar_mul(wq_sb[:, kc, :], wq_sb[:, kc, :],
                                        gcol[:, kc:kc + 1])

        kT = kvpool.tile([128, 8, TK], F16)
        v_sb = kvpool.tile([128, 2, HEADS, HD + 1], F16)
        nc.vector.tensor_copy(
            v_sb[:, :, :, HD:HD + 1],
            onesf[:, 0:1].unsqueeze(1).unsqueeze(1).broadcast_to([128, 2, HEADS, 1]))
        vnull = kvpool.tile([1, HD + 1], F16)

        # ---------------- phase K: context -> kT, v' ----------------
        with ExitStack() as kctx:
            pk = kctx.enter_context(tc.tile_pool(name="pk", bufs=2))
            pk1 = kctx.enter_context(tc.tile_pool(name="pk1", bufs=1))

            cnT = pk1.tile([128, 8, T], F16)
            for i in range(2):
                ctx_t = pk.tile([128, DIM], F32, tag="ctx")
                nc.sync.dma_start(out=ctx_t[:], in_=CTX[i * 128:(i + 1) * 128, :])
                rstd, negmr = _emit_ln_stats(
                    nc, small, [ctx_t[:, 0:512], ctx_t[:, 512:1024]], eps_tile)
                cnf = pk.tile([128, DIM], F32, tag="cnf")
                nc.scalar.activation(out=cnf[:], in_=ctx_t[:], func=AF.Identity,
                                     bias=negmr[:], scale=rstd[:])
                nc.vector.tensor_mul(cnf[:], cnf[:], lcg[:])
                cn = pk.tile([128, DIM], F16, tag="cn")
                nc.vector.tensor_add(cn[:], cnf[:], lcb[:])
                ptr = ps_mi.tile([128, 8, 128], F16, tag="mi")
                for t in range(8):
                    nc.tensor.transpose(ptr[:, t, :],
                                        cn[:, t * 128:(t + 1) * 128],
                                        ident[:])
                nc.vector.tensor_copy(cnT[:, :, i * 128:(i + 1) * 128], ptr[:])

            for i in range(2):
                for which in (0, 1):  # 0 = k, 1 = v
                    ph = [ps_half.tile([128, 512], F32, tag="ph", name=f"ph{_i}")
                          for _i in range(2)]
                    for half in range(2):
                        for kc in range(8):
                            wkv_c = pk.tile([128, 512], F16, tag="wkv")
                            nc.sync.dma_start(
                                out=wkv_c[:],
                                in_=WKV[kc * 128:(kc + 1) * 128,
                                        which * DIM + half * 512:
                                        which * DIM + (half + 1) * 512])
                            nc.tensor.matmul(
                                ph[half][:],
                                lhsT=cnT[:, kc, i * 128:(i + 1) * 128],
                                rhs=wkv_c[:],
                                start=(kc == 0), stop=(kc == 7))
                    if which == 0:
                        kfin = pk.tile([128, DIM], F16, tag="kfin")
                        _emit_l2norm_heads(
                            nc, small, sq_pool,
                            kfin[:].rearrange("p (h d) -> p h d", d=HD),
                            [ph[0][:], ph[1][:]], kqsc)
                        ptr = ps_mi.tile([128, 8, 128], F16, tag="mi")
                        for t in range(8):
                            nc.tensor.transpose(
                                ptr[:, t, :],
                                kfin[:, t * 128:(t + 1) * 128],
                                ident[:])
                        nc.vector.tensor_copy(
                            kT[:, :, i * 128:(i + 1) * 128], ptr[:])
                    else:
                        for half in range(2):
                            nc.vector.tensor_copy(
                                v_sb[:, i, half * 8:(half + 1) * 8, 0:HD],
                                ph[half][:].rearrange("p (h d) -> p h d", d=HD))

            # null kv token
            nkv = pk1.tile([1, 2, HD], F32)
            nc.sync.dma_start(out=nkv[0:1, :, :], in_=NKV[:, :])
            sqn = pk1.tile([1, HD], F32)
            nc.vector.tensor_mul(sqn[0:1, :], nkv[0:1, 0, :], nkv[0:1, 0, :])
            ssqn = pk1.tile([1, 1], F32)
            nc.vector.reduce_sum(out=ssqn[0:1, :], in_=sqn[0:1, :], axis=AX.X)
            nc.scalar.activation(out=ssqn[0:1, :], in_=ssqn[0:1, :], func=AF.Sqrt,
                                 bias=0.0, scale=1.0)
            nc.vector.tensor_scalar_max(ssqn[0:1, :], ssqn[0:1, :], NORM_EPS)
            rnn = pk1.tile([1, 1], F32)
            nc.vector.reciprocal(rnn[0:1, :], ssqn[0:1, :])
            knf = pk1.tile([1, HD], F32)
            nc.vector.tensor_mul(knf[0:1, :], nkv[0:1, 0, :],
                                 rnn[0:1, 0:1].broadcast_to([1, HD]))
            nc.vector.tensor_mul(knf[0:1, :], knf[0:1, :], kqsc[0:1, :])
            kn16 = pk1.tile([1, HEADS, HD], F16)
            nc.vector.tensor_copy(kn16[0:1, :, :],
                                  knf[0:1, :].unsqueeze(1).broadcast_to([1, HEADS, HD]))
            kn16f = kn16[0:1, :, :].rearrange("p h d -> p (h d)")
            ptr = ps_mi.tile([128, 8, 2], F16, tag="mi")
            for c in range(8):
                nc.tensor.transpose(ptr[:, c, 0:1],
                                    kn16f[0:1, c * 128:(c + 1) * 128],
                                    ident[0:1, 0:1])
            nc.vector.tensor_copy(kT[:, :, T:T + 1], ptr[:, :, 0:1])
            nc.vector.tensor_copy(vnull[0:1, 0:HD], nkv[0:1, 1, :])
            nc.vector.tensor_copy(vnull[0:1, HD:HD + 1], onesf[0:1, 0:1])

        # ---------------- main loop pools ----------------
        xin = ctx.enter_context(tc.tile_pool(name="xin", bufs=4))
        xnp = ctx.enter_context(tc.tile_pool(name="xnp", bufs=6))
        xnTp = ctx.enter_context(tc.tile_pool(name="xnTp", bufs=2))
        qfp = ctx.enter_context(tc.tile_pool(name="qfp", bufs=2))
        qTp = ctx.enter_context(tc.tile_pool(name="qTp", bufs=2))
        etp = ctx.enter_context(tc.tile_pool(name="etp", bufs=2))
        etn = ctx.enter_context(tc.tile_pool(name="etn", bufs=2))
        sumsp = ctx.enter_context(tc.tile_pool(name="sumsp", bufs=2))
        rbp = ctx.enter_context(tc.tile_pool(name="rbp", bufs=2))
        stgp = ctx.enter_context(tc.tile_pool(name="stgp", bufs=2))
        outTp = ctx.enter_context(tc.tile_pool(name="outTp", bufs=2))
        outp = ctx.enter_context(tc.tile_pool(name="outp", bufs=2))

        for m in range(NMACRO):
            qT = qTp.tile([128, 8, QMACRO], F16, tag="qT")
            # ---- LN phase: all 4 subtiles' serial LN chains up front ----
            xns = []
            for s in range(NSUB):
                r0 = m * QMACRO + s * 128
                x_t = xin.tile([128, DIM], F32, tag="x")
                nc.sync.dma_start(out=x_t[:], in_=XS[r0:r0 + 128, :])
                rstd, negmr = _emit_ln_stats(
                    nc, small, [x_t[:, 0:512], x_t[:, 512:1024]], eps_tile)
                xn = xnp.tile([128, DIM], F16, tag="xn")
                nc.scalar.activation(out=xn[:], in_=x_t[:], func=AF.Identity,
                                     bias=negmr[:], scale=rstd[:])
                xns.append(xn)
            # ---- PE phase per subtile ----
            for s in range(NSUB):
                xn = xns[s]
                xnT = xnTp.tile([128, 8, 128], F16, tag="xnT")
                ptr = ps_mi.tile([128, 8, 128], F16, tag="mi")
                for t in range(8):
                    nc.tensor.transpose(ptr[:, t, :],
                                        xn[:, t * 128:(t + 1) * 128],
                                        ident[:])
                nc.vector.tensor_copy(xnT[:], ptr[:])
                pq = [ps_half.tile([128, 512], F32, tag="ph", name=f"pq{_i}")
                      for _i in range(2)]
                for half in range(2):
                    nc.tensor.matmul(pq[half][:], lhsT=ones_row[0:1, :],
                                     rhs=qbias[0:1, half * 512:(half + 1) * 512],
                                     start=True, stop=False)
                    for kc in range(8):
                        nc.tensor.matmul(
                            pq[half][:],
                            lhsT=xnT[:, kc, :],
                            rhs=wq_sb[:, kc, half * 512:(half + 1) * 512],
                            start=False, stop=(kc == 7))
                qf = qfp.tile([128, DIM], F16, tag="qf")
                _emit_l2norm_heads(nc, small, sq_pool,
                                   qf[:].rearrange("p (h d) -> p h d", d=HD),
                                   [pq[0][:], pq[1][:]], None)
                ptr = ps_mi.tile([128, 8, 128], F16, tag="mi")
                for t in range(8):
                    nc.tensor.transpose(ptr[:, t, :],
                                        qf[:, t * 128:(t + 1) * 128],
                                        ident[:])
                nc.vector.tensor_copy(qT[:, :, s * 128:(s + 1) * 128], ptr[:])

            # ---- head stage ----
            outT = outTp.tile([128, 8, QMACRO], F16, tag="outT")
            for h in range(HEADS):
                c, j = h // 2, h % 2
                jb = j * HD
                kT_h = kT[jb:jb + HD, c, :]
                qT_h = qT[jb:jb + HD, c, :]
                ps_s = ps_sc.tile([128, 2, QMACRO], F32, tag="sc")
                for kc in range(2):
                    nc.tensor.matmul(ps_s[:, kc, :],
                                     lhsT=kT_h[:, kc * 128:(kc + 1) * 128],
                                     rhs=qT_h, start=True, stop=True)
                ps_n = ps_mi.tile([1, QMACRO], F32, tag="mi")
                nc.tensor.matmul(ps_n[:], lhsT=kT_h[:, T:T + 1],
                                 rhs=qT_h, start=True, stop=True)
                et = etp.tile([128, 2, QMACRO], F16, tag="et")
                for kc in range(2):
                    nc.scalar.activation(out=et[:, kc, :], in_=ps_s[:, kc, :],
                                         func=AF.Exp, bias=expb[:], scale=SCALE)
                en = etn.tile([1, QMACRO], F16, tag="en")
                nc.scalar.activation(out=en[0:1, :], in_=ps_n[0:1, :],
                                     func=AF.Exp, bias=expb[0:1, :], scale=SCALE)
                po = ps_o.tile([HD + 1, QMACRO], F32, tag="o")
                nc.tensor.matmul(po[:], lhsT=v_sb[:, 0, h, :],
                                 rhs=et[:, 0, :], start=True, stop=False)
                nc.tensor.matmul(po[:], lhsT=v_sb[:, 1, h, :],
                                 rhs=et[:, 1, :], start=False, stop=False)
                nc.tensor.matmul(po[:], lhsT=vnull[0:1, :],
                                 rhs=en[0:1, :], start=False, stop=True)
                sums = sumsp.tile([HD + 1, QMACRO], F16, tag="sums")
                if j == 0:
                    nc.vector.tensor_copy(sums[HD:HD + 1, :], po[HD:HD + 1, :])
                else:
                    nc.scalar.copy(sums[HD:HD + 1, :], po[HD:HD + 1, :])
                pb = ps_mi.tile([HD, QMACRO], F32, tag="mi")
                nc.tensor.matmul(pb[:], lhsT=ones_t[HD:HD + 1, 0:HD],
                                 rhs=sums[HD:HD + 1, :], start=True, stop=True)
                rb = rbp.tile([HD, QMACRO], F32, tag="rb")
                nc.vector.reciprocal(rb[:], pb[:])
                if j == 0:
                    nc.vector.tensor_mul(outT[0:HD, c, :], po[0:HD, :], rb[:])
                else:
                    stg = stgp.tile([HD, QMACRO], F16, tag="stg")
                    nc.vector.tensor_mul(stg[:], po[0:HD, :], rb[:])
                    nc.sync.dma_start(out=outT[HD:128, c, :], in_=stg[:])

            # ---- output stage: Wo + LN out ----
            for s in range(NSUB):
                r0 = m * QMACRO + s * 128
                pf = [ps_half.tile([128, 512], F32, tag="ph", name=f"pf{_i}")
                      for _i in range(2)]
                for half in range(2):
                    for kc in range(8):
                        nc.tensor.matmul(
                            pf[half][:],
                            lhsT=outT[:, kc, s * 128:(s + 1) * 128],
                            rhs=wo_sb[:, kc, half * 512:(half + 1) * 512],
                            start=(kc == 0), stop=(kc == 7))
                rstd, negmr = _emit_ln_stats(nc, small, [pf[0][:], pf[1][:]],
                                             eps_tile)
                ob = outp.tile([128, DIM], F32, tag="ob")
                for half in range(2):
                    nc.scalar.activation(out=ob[:, half * 512:(half + 1) * 512],
                                         in_=pf[half][:], func=AF.Identity,
                                         bias=negmr[:], scale=rstd[:])
                nc.vector.tensor_mul(ob[:], ob[:], log_[:])
                nc.vector.tensor_add(ob[:], ob[:], lob[:])
                nc.sync.dma_start(out=OUT[r0:r0 + 128, :], in_=ob[:])

    nc.compile()
    return nc


_NC_CACHE = None


def kernel(**inputs):
    global _NC_CACHE
    if _NC_CACHE is None:
        _NC_CACHE = build_nc()
    nc = _NC_CACHE

    x = np.asarray(inputs["x"], np.float32)
    context = np.asarray(inputs["context"], np.float32)
    shared = {
        "Wq": np.asarray(inputs["Wq"], np.float32).astype(np.float16),
        "Wkv": np.asarray(inputs["Wkv"], np.float32).astype(np.float16),
        "Wo": np.asarray(inputs["Wo"], np.float32).astype(np.float16),
        "null_kv": np.asarray(inputs["null_kv"], np.float32),
        "q_scale": np.asarray(inputs["q_scale"], np.float32),
        "k_scale": np.asarray(inputs["k_scale"], np.float32),
        "ln_in_g": np.asarray(inputs["ln_in_g"], np.float32),
        "ln_in_b": np.asarray(inputs["ln_in_b"], np.float32),
        "ln_ctx_g": np.asarray(inputs["ln_ctx_g"], np.float32),
        "ln_ctx_b": np.asarray(inputs["ln_ctx_b"], np.float32),
        "ln_out_g": np.asarray(inputs["ln_out_g"], np.float32),
        "ln_out_b": np.asarray(inputs["ln_out_b"], np.float32),
    }
    B, N, _ = x.shape
    in_maps = []
    for c in range(N_CORES):
        b, n0 = c // 2, (c % 2) * ROWS
        in_maps.append({"xs": np.ascontiguousarray(x[b, n0:n0 + ROWS]),
                        "ctx": np.ascontiguousarray(context[b]), **shared})

    res = run_bass_kernel_spmd(nc, in_maps, list(range(N_CORES)))

    out = np.empty((B, N, DIM), np.float32)
    for c in range(N_CORES):
        b, n0 = c // 2, (c % 2) * ROWS
        out[b, n0:n0 + ROWS] = res.results[c]["out"]
    return out



# revision 14
# speedup vs baseline: 1.1164x; 1.1164x over previous
"""Trainium2 Bass kernel for nn_CrossAttention (B=4, N=4096, T=256, DIM=1024,
16 heads x 64 dim, cosine-sim attention with null-kv token, LN in/ctx/out).

Sharding: data-parallel over query rows. Core c handles batch b=c//2, query
rows (c%2)*2048 : (c%2)*2048+2048. The kv projections (tiny: T=256) are
computed redundantly per core; no collectives are needed.

v2 rewrite (cost-model driven):
  - Null-kv scores computed as 16 extra q-projection columns (wq_null =
    Wq'_head-blocks @ (k_null_hat*qs*ks), rank-16) instead of a per-head
    K=1 matmul against a kT null column: kills 64 N=512 PE matmuls and 64
    single-partition Act exps; the null exp becomes one [16, 512] exp/macro.
  - Softmax denominator: reciprocal of the ones-column row of po (PSUM)
    into an SBUF row, partition-broadcast on the idle GPSIMD engine, then
    one aligned DVE multiply. Odd heads get v' column-padded ([0*63, 1, v])
    so their po lands on partitions 63..127 directly: no partition-shift
    DMA, no [64,512] broadcast matmul (pb), no PSUM->SBUF sums copies.
  - rsqrt via exp(-0.5*ln(x)) so every Act function (Identity/Square/Exp/
    Ln/Copy) lives in the natural_log_exp_and_others table: zero
    ACT_TABLE_LOAD switches (LN row-scale precision cancels in the cosine
    attention anyway).
  - All weights (incl. Wkv) prefetched; next macro's x-LayerNorm is issued
    before this macro's head stage so PE never waits on the serial LN chain
    (keeps the Tensor engine p-state ramped at 2.4 GHz).
  - fp16 squares/reduces for the l2 norms (2x/4x DVE modes); ln_out bias-add
    moved to GPSIMD; gain-mul done in fp16.
"""

import numpy as np
from contextlib import ExitStack

import concourse.bass as bass
import concourse.tile as tile
from concourse import bacc, mybir
from concourse.bass_utils import run_bass_kernel_spmd
from concourse.masks import make_identity

F32 = mybir.dt.float32
F16 = mybir.dt.float16
AF = mybir.ActivationFunctionType
AX = mybir.AxisListType
ALU = mybir.AluOpType

DIM = 1024
HEADS = 16
HD = 64
T = 256
SCALE = 8.0
EXPB = -5.545177444479562  # ln(1/256)
LN_EPS = 1e-5
N_CORES = 8
ROWS = 2048
QMACRO = 512
NSUB = QMACRO // 128
NMACRO = ROWS // QMACRO


def _emit_ln_stats(nc, pool_small, in_aps, eps_tile):
    """in_aps: list of [128, 512] APs covering the 1024 row. Returns
    (rstd [128,1], negmr [128,1]) fp32 tiles for (x - m) * rstd.
    rstd = exp(-0.5 * ln(var + eps)) to stay in the exp act table."""
    stats = pool_small.tile([128, 2, 6], F32, tag="lnstats")
    for i, ap in enumerate(in_aps):
        nc.vector.bn_stats(out=stats[:, i, :], in_=ap)
    mv = pool_small.tile([128, 2], F32, tag="lnmv")
    nc.vector.bn_aggr(out=mv[:], in_=stats[:])
    lnv = pool_small.tile([128, 1], F32, tag="lnlnv")
    nc.scalar.activation(out=lnv[:], in_=mv[:, 1:2], func=AF.Ln,
                         bias=eps_tile[:], scale=1.0)
    rstd = pool_small.tile([128, 1], F32, tag="lnrstd")
    nc.scalar.activation(out=rstd[:], in_=lnv[:], func=AF.Exp,
                         bias=0.0, scale=-0.5)
    negmr = pool_small.tile([128, 1], F32, tag="lnnegmr")
    nc.vector.scalar_tensor_tensor(out=negmr[:], in0=mv[:, 0:1], scalar=-1.0,
                                   in1=rstd[:], op0=ALU.mult, op1=ALU.mult)
    return rstd, negmr


def _emit_l2norm_heads(nc, pool_small, sq_pool, out_ap3, in_half_aps, scale_tile, tiny):
    """in_half_aps: [pa, pb] fp32 psum APs [128, 512] (heads 0-7, 8-15).
    out_ap3: [128, 16, 64] fp16 sbuf AP. out = in * rsqrt(ssq_head)
    (* scale_tile [128,64] if given). Returns rn [128, 16] fp32."""
    sq = sq_pool.tile([128, 1024], F16, tag="sq")
    nc.scalar.activation(out=sq[:, 0:512], in_=in_half_aps[0], func=AF.Square,
                         bias=0.0, scale=1.0)
    nc.scalar.activation(out=sq[:, 512:1024], in_=in_half_aps[1], func=AF.Square,
                         bias=0.0, scale=1.0)
    ssq = pool_small.tile([128, 16], F16, tag="ssq")
    with nc.allow_low_precision(reason="l2norm ssq in fp16; rel err ~1e-3 ok"):
        nc.vector.reduce_sum(out=ssq[:], in_=sq[:].rearrange("p (h d) -> p h d", d=HD),
                             axis=AX.X)
    lns = pool_small.tile([128, 16], F32, tag="l2ln")
    nc.scalar.activation(out=lns[:], in_=ssq[:], func=AF.Ln,
                         bias=tiny[:], scale=1.0)
    rn = pool_small.tile([128, 16], F32, tag="l2rn")
    nc.scalar.activation(out=rn[:], in_=lns[:], func=AF.Exp,
                         bias=0.0, scale=-0.5)
    for i in range(2):
        h0 = i * 8
        out_h = out_ap3[:, h0:h0 + 8, :]
        in3 = in_half_aps[i].rearrange("p (h d) -> p h d", d=HD)
        nc.vector.tensor_mul(
            out_h, in3,
            rn[:, h0:h0 + 8].unsqueeze(-1).broadcast_to([128, 8, HD]))
        if scale_tile is not None:
            nc.vector.tensor_mul(
                out_h, out_h,
                scale_tile[:].unsqueeze(1).broadcast_to([128, 8, HD]))
    return rn


def _load_bcast(nc, dst_tile, dram_ap, parts=128):
    ap = bass.AP(tensor=dram_ap.tensor, offset=dram_ap.offset,
                 ap=[[0, parts]] + dram_ap.ap)
    nc.sync.dma_start(out=dst_tile[:parts, :], in_=ap)


def _load_cols(nc, dst_tile, dram_ap):
    """Load a [1024] dram vector as [128, 8] (dst[p, kc] = v[kc*128+p])."""
    ap = bass.AP(tensor=dram_ap.tensor, offset=dram_ap.offset,
                 ap=[[1, 128], [128, 8]])
    nc.sync.dma_start(out=dst_tile[:, :], in_=ap)


def build_nc():
    nc = bacc.Bacc("TRN2", debug=False)

    XS = nc.dram_tensor("xs", [ROWS, DIM], F32, kind="ExternalInput")
    CTX = nc.dram_tensor("ctx", [T, DIM], F32, kind="ExternalInput")
    WQ = nc.dram_tensor("Wq", [DIM, DIM], F16, kind="ExternalInput")
    WKV = nc.dram_tensor("Wkv", [DIM, 2 * DIM], F16, kind="ExternalInput")
    WO = nc.dram_tensor("Wo", [DIM, DIM], F16, kind="ExternalInput")
    NKV = nc.dram_tensor("null_kv", [2, HD], F32, kind="ExternalInput")
    QS = nc.dram_tensor("q_scale", [HD], F32, kind="ExternalInput")
    KS = nc.dram_tensor("k_scale", [HD], F32, kind="ExternalInput")
    LIG = nc.dram_tensor("ln_in_g", [DIM], F32, kind="ExternalInput")
    LIB = nc.dram_tensor("ln_in_b", [DIM], F32, kind="ExternalInput")
    LCG = nc.dram_tensor("ln_ctx_g", [DIM], F32, kind="ExternalInput")
    LCB = nc.dram_tensor("ln_ctx_b", [DIM], F32, kind="ExternalInput")
    LOG = nc.dram_tensor("ln_out_g", [DIM], F32, kind="ExternalInput")
    LOB = nc.dram_tensor("ln_out_b", [DIM], F32, kind="ExternalInput")
    OUT = nc.dram_tensor("out", [ROWS, DIM], F32, kind="ExternalOutput")

    with tile.TileContext(nc) as tc, ExitStack() as ctx:
        consts = ctx.enter_context(tc.tile_pool(name="consts", bufs=1))
        weights = ctx.enter_context(tc.tile_pool(name="weights", bufs=1))
        kvpool = ctx.enter_context(tc.tile_pool(name="kvpool", bufs=1))
        small = ctx.enter_context(tc.tile_pool(name="small", bufs=3))
        sq_pool = ctx.enter_context(tc.tile_pool(name="sqp", bufs=2))

        # PSUM: half(2) + mi(2) + sc(2) + oe(1) + oo(1) = 8 banks
        ps_half = ctx.enter_context(tc.tile_pool(name="ps_half", bufs=2, space="PSUM"))
        ps_mi = ctx.enter_context(tc.tile_pool(name="ps_mi", bufs=2, space="PSUM"))
        ps_sc = ctx.enter_context(tc.tile_pool(name="ps_sc", bufs=2, space="PSUM"))
        ps_oe = ctx.enter_context(tc.tile_pool(name="ps_oe", bufs=1, space="PSUM"))
        ps_oo = ctx.enter_context(tc.tile_pool(name="ps_oo", bufs=1, space="PSUM"))

        # ---------------- input DMAs, highest priority first ----------------
        nkvb = consts.tile([128, 2, HD], F32)
        _load_bcast(nc, nkvb.rearrange("p a b -> p (a b)"),
                    NKV[:, :].rearrange("a b -> (a b)"))
        lcg = kvpool.tile([128, DIM], F32)
        lcb = kvpool.tile([128, DIM], F32)
        _load_bcast(nc, lcg, LCG[:])
        _load_bcast(nc, lcb, LCB[:])
        qsc = consts.tile([128, HD], F32)
        _load_bcast(nc, qsc, QS[:])
        ksc = consts.tile([128, HD], F32)
        _load_bcast(nc, ksc, KS[:])

        xin = ctx.enter_context(tc.tile_pool(name="xin", bufs=6))
        xnp = ctx.enter_context(tc.tile_pool(name="xnp", bufs=8))
        wkv_ctx = ExitStack()
        wkvp = wkv_ctx.enter_context(tc.tile_pool(name="wkvp", bufs=1))
        wkv_sb = wkvp.tile([128, 8, 2 * DIM], F16)
        for kc in range(8):
            nc.sync.dma_start(out=wkv_sb[:, kc, :],
                              in_=WKV[kc * 128:(kc + 1) * 128, :])
        ctx_t = []
        for i in range(2):
            ct = kvpool.tile([128, DIM], F32, tag=f"ctx{i}", name=f"ctx{i}")
            nc.sync.dma_start(out=ct[:], in_=CTX[i * 128:(i + 1) * 128, :])
            ctx_t.append(ct)

        x0_tiles = []
        for s in range(NSUB):
            x_t = xin.tile([128, DIM], F32, tag="x")
            nc.sync.dma_start(out=x_t[:], in_=XS[s * 128:(s + 1) * 128, :])
            x0_tiles.append(x_t)

        gcol = consts.tile([128, 8], F32)
        _load_cols(nc, gcol, LIG[:])
        bcol = consts.tile([128, 8], F32)
        _load_cols(nc, bcol, LIB[:])

        wq_sb = weights.tile([128, 8, DIM], F16)
        for kc in range(8):
            nc.sync.dma_start(out=wq_sb[:, kc, :], in_=WQ[kc * 128:(kc + 1) * 128, :])

        log32 = consts.tile([128, DIM], F32)
        _load_bcast(nc, log32, LOG[:])
        lob = consts.tile([128, DIM], F32)
        _load_bcast(nc, lob, LOB[:])
        log16 = consts.tile([128, DIM], F16)
        nc.vector.tensor_copy(log16[:], log32[:])

        wo_sb = weights.tile([128, 8, DIM], F16)
        for kc in range(8):
            nc.sync.dma_start(out=wo_sb[:, kc, :], in_=WO[kc * 128:(kc + 1) * 128, :])

        # ---------------- small consts ----------------
        ident = consts.tile([128, 128], F16)
        make_identity(nc, ident)
        eps_tile = consts.tile([128, 1], F32)
        nc.vector.memset(eps_tile[:], LN_EPS)
        tiny_tile = consts.tile([128, 1], F32)
        nc.vector.memset(tiny_tile[:], 1e-12)
        onesf = consts.tile([128, 1], F32)
        nc.vector.memset(onesf[:], 1.0)
        expb = consts.tile([128, 1], F32)
        nc.vector.memset(expb[:], EXPB)
        ones64 = consts.tile([128, HD], F16)
        nc.vector.memset(ones64[:], 1.0)
        ones_row = consts.tile([1, 128], F16)
        nc.vector.tensor_copy(ones_row[0:1, :], onesf[0:1, 0:1].broadcast_to([1, 128]))
        kqsc = consts.tile([128, HD], F32)
        nc.vector.tensor_mul(kqsc[:], ksc[:], qsc[:])
        bcol16 = consts.tile([128, 8], F16)
        nc.vector.tensor_copy(bcol16[:], bcol[:])

        # ---------------- null-kv prep (all 128 partitions in parallel) -----
        # w = k_null_hat * (qs*ks)  [128, 64] f32 (same value on every row)
        sqn = consts.tile([128, HD], F32)
        nc.vector.tensor_mul(sqn[:], nkvb[:, 0, :], nkvb[:, 0, :])
        ssqn = consts.tile([128, 1], F32)
        nc.vector.reduce_sum(out=ssqn[:], in_=sqn[:], axis=AX.X)
        lnn = consts.tile([128, 1], F32)
        nc.scalar.activation(out=lnn[:], in_=ssqn[:], func=AF.Ln,
                             bias=tiny_tile[:], scale=1.0)
        rnn = consts.tile([128, 1], F32)
        nc.scalar.activation(out=rnn[:], in_=lnn[:], func=AF.Exp,
                             bias=0.0, scale=-0.5)
        w_b = consts.tile([128, HD], F16)
        nc.vector.scalar_tensor_tensor(out=w_b[:], in0=nkvb[:, 0, :], scalar=rnn[:],
                                       in1=kqsc[:], op0=ALU.mult, op1=ALU.mult)
        # per-head masked null-v weights (K=16, base partition 0):
        # vne_m[p, h, :] = (p==h) ? [v_null, 1] : 0   [16, 16, 65]
        # vno_m[p, h, :] = (p==h) ? [1, 0*63, v_null] : 0 [16, 16, 128]
        # (odd-head sums row sits at partition 0: partition_broadcast only
        #  works base-0 -> base-0 on hardware)
        vrow_e = consts.tile([128, HD + 1], F16)
        nc.vector.tensor_copy(vrow_e[:, 0:HD], nkvb[:, 1, :])
        nc.vector.tensor_copy(vrow_e[:, HD:HD + 1], onesf[:, 0:1])
        vrow_o = consts.tile([128, 128], F16)
        nc.vector.memset(vrow_o[:, 0:64], 0.0)
        nc.vector.tensor_copy(vrow_o[:, 0:1], onesf[:, 0:1])
        nc.vector.tensor_copy(vrow_o[:, 64:128], nkvb[:, 1, :])
        vne_m = kvpool.tile([16, HEADS, HD + 1], F16)
        nc.vector.tensor_mul(
            vne_m[:],
            ident[0:16, 0:16].unsqueeze(-1).broadcast_to([16, HEADS, HD + 1]),
            vrow_e[0:16, :].unsqueeze(1).broadcast_to([16, HEADS, HD + 1]))
        vno_m = kvpool.tile([16, HEADS, 128], F16)
        nc.vector.tensor_mul(
            vno_m[:],
            ident[0:16, 0:16].unsqueeze(-1).broadcast_to([16, HEADS, 128]),
            vrow_o[0:16, :].unsqueeze(1).broadcast_to([16, HEADS, 128]))

        # ---------------- phase K: context -> kT, v_e, v_o ----------------
        kT = kvpool.tile([128, 8, T], F16)
        v_e = kvpool.tile([128, 2, 8, HD + 1], F16)
        nc.vector.tensor_copy(
            v_e[:, :, :, HD:HD + 1],
            onesf[:, 0:1].unsqueeze(1).unsqueeze(1).broadcast_to([128, 2, 8, 1]))
        v_o = kvpool.tile([128, 2, 8, 128], F16)
        nc.vector.memset(v_o[:, :, :, 0:64], 0.0)
        nc.vector.tensor_copy(
            v_o[:, :, :, 0:1],
            onesf[:, 0:1].unsqueeze(1).unsqueeze(1).broadcast_to([128, 2, 8, 1]))

        with ExitStack() as kctx:
            pk = kctx.enter_context(tc.tile_pool(name="pk", bufs=2))
            pk1 = kctx.enter_context(tc.tile_pool(name="pk1", bufs=1))

            cnT = pk1.tile([128, 8, T], F16)
            for i in range(2):
                rstd, negmr = _emit_ln_stats(
                    nc, small, [ctx_t[i][:, 0:512], ctx_t[i][:, 512:1024]], eps_tile)
                cnf = pk.tile([128, DIM], F32, tag="cnf")
                nc.scalar.activation(out=cnf[:], in_=ctx_t[i][:], func=AF.Identity,
                                     bias=negmr[:], scale=rstd[:])
                nc.vector.tensor_mul(cnf[:], cnf[:], lcg[:])
                cn = pk.tile([128, DIM], F16, tag="cn")
                nc.vector.tensor_add(cn[:], cnf[:], lcb[:])
                ptr = ps_mi.tile([128, 8, 128], F16, tag="mi")
                for t in range(8):
                    nc.tensor.transpose(ptr[:, t, :],
                                        cn[:, t * 128:(t + 1) * 128],
                                        ident[:])
                nc.vector.tensor_copy(cnT[:, :, i * 128:(i + 1) * 128], ptr[:])

            for i in range(2):
                for which in (0, 1):  # 0 = k, 1 = v
                    ph = [ps_half.tile([128, 512], F32, tag="ph", name=f"ph{_i}")
                          for _i in range(2)]
                    for half in range(2):
                        for kc in range(8):
                            nc.tensor.matmul(
                                ph[half][:],
                                lhsT=cnT[:, kc, i * 128:(i + 1) * 128],
                                rhs=wkv_sb[:, kc,
                                           which * DIM + half * 512:
                                           which * DIM + (half + 1) * 512],
                                start=(kc == 0), stop=(kc == 7))
                    if which == 0:
                        kfin = pk.tile([128, DIM], F16, tag="kfin")
                        _emit_l2norm_heads(
                            nc, small, sq_pool,
                            kfin[:].rearrange("p (h d) -> p h d", d=HD),
                            [ph[0][:], ph[1][:]], kqsc, tiny_tile)
                        ptr = ps_mi.tile([128, 8, 128], F16, tag="mi")
                        for t in range(8):
                            nc.tensor.transpose(
                                ptr[:, t, :],
                                kfin[:, t * 128:(t + 1) * 128],
                                ident[:])
                        nc.vector.tensor_copy(
                            kT[:, :, i * 128:(i + 1) * 128], ptr[:])
                    else:
                        for half in range(2):
                            ph3 = ph[half][:].rearrange("p (h d) -> p h d", d=HD)
                            for hh in range(8):
                                h = half * 8 + hh
                                c = h // 2
                                if h % 2 == 0:
                                    nc.vector.tensor_copy(
                                        v_e[:, i, c, 0:HD], ph3[:, hh, :])
                                else:
                                    nc.vector.tensor_copy(
                                        v_o[:, i, c, 64:128], ph3[:, hh, :])

            # qbias = ln_in_b @ Wq (rank-1), before Wq is gain-scaled
            qbias = kvpool.tile([1, DIM], F16)
            for half in range(2):
                pqb = ps_mi.tile([1, 512], F32, tag="mi")
                for kc in range(8):
                    nc.tensor.matmul(pqb[:], lhsT=bcol16[:, kc:kc + 1],
                                     rhs=wq_sb[:, kc, half * 512:(half + 1) * 512],
                                     start=(kc == 0), stop=(kc == 7))
                nc.vector.tensor_copy(qbias[0:1, half * 512:(half + 1) * 512], pqb[:])
            # Wq' = diag(ln_in_g) @ Wq
            for kc in range(8):
                nc.vector.tensor_scalar_mul(wq_sb[:, kc, :], wq_sb[:, kc, :],
                                            gcol[:, kc:kc + 1])
            # wq_null[:, kc, h] = sum_e Wq'[:, kc, h*64+e] * w[e]
            wq_null = weights.tile([128, 8, HEADS], F16)
            for kc in range(8):
                tmp = sq_pool.tile([128, 1024], F16, tag="sq")
                nc.vector.tensor_mul(
                    tmp[:].rearrange("p (h d) -> p h d", d=HD),
                    wq_sb[:, kc, :].rearrange("p (h d) -> p h d", d=HD),
                    w_b[:].unsqueeze(1).broadcast_to([128, HEADS, HD]))
                with nc.allow_low_precision(reason="rank-16 null cols fp16"):
                    nc.vector.reduce_sum(
                        out=wq_null[:, kc, :],
                        in_=tmp[:].rearrange("p (h d) -> p h d", d=HD), axis=AX.X)
            # qbias_null[h] = sum_e qbias[h*64+e] * w[e]
            qbn_t = pk1.tile([1, DIM], F16)
            nc.vector.tensor_mul(
                qbn_t[0:1, :].rearrange("p (h d) -> p h d", d=HD),
                qbias[0:1, :].rearrange("p (h d) -> p h d", d=HD),
                w_b[0:1, :].unsqueeze(1).broadcast_to([1, HEADS, HD]))
            qbias_null = kvpool.tile([1, HEADS], F16)
            with nc.allow_low_precision(reason="null bias fp16"):
                nc.vector.reduce_sum(
                    out=qbias_null[0:1, :],
                    in_=qbn_t[0:1, :].rearrange("p (h d) -> p h d", d=HD), axis=AX.X)
        wkv_ctx.close()

        # ---------------- main loop pools ----------------
        xnTp = ctx.enter_context(tc.tile_pool(name="xnTp", bufs=2))
        qfp = ctx.enter_context(tc.tile_pool(name="qfp", bufs=2))
        qTp = ctx.enter_context(tc.tile_pool(name="qTp", bufs=2))
        etp = ctx.enter_context(tc.tile_pool(name="etp", bufs=4))
        rp = ctx.enter_context(tc.tile_pool(name="rp", bufs=2))
        rbcp = ctx.enter_context(tc.tile_pool(name="rbcp", bufs=2))
        enp = ctx.enter_context(tc.tile_pool(name="enp", bufs=2))
        nsp = ctx.enter_context(tc.tile_pool(name="nsp", bufs=4))
        outTp = ctx.enter_context(tc.tile_pool(name="outTp", bufs=2))
        obp = ctx.enter_context(tc.tile_pool(name="obp", bufs=2))
        obgp = ctx.enter_context(tc.tile_pool(name="obgp", bufs=2))
        obfp = ctx.enter_context(tc.tile_pool(name="obfp", bufs=2))

        def issue_x_ln(m, x_tiles=None):
            """DMA + LayerNorm for macro m's 4 subtiles. Returns xn tiles."""
            xns = []
            for s in range(NSUB):
                r0 = m * QMACRO + s * 128
                if x_tiles is not None:
                    x_t = x_tiles[s]
                else:
                    x_t = xin.tile([128, DIM], F32, tag="x")
                    nc.sync.dma_start(out=x_t[:], in_=XS[r0:r0 + 128, :])
                rstd, negmr = _emit_ln_stats(
                    nc, small, [x_t[:, 0:512], x_t[:, 512:1024]], eps_tile)
                xn = xnp.tile([128, DIM], F16, tag="xn")
                nc.scalar.activation(out=xn[:], in_=x_t[:], func=AF.Identity,
                                     bias=negmr[:], scale=rstd[:])
                xns.append(xn)
            return xns

        xns_cur = issue_x_ln(0, x0_tiles)

        for m in range(NMACRO):
            # ---- B phase: transposes + q projection + l2norm + null cols ----
            qT = qTp.tile([128, 8, QMACRO], F16, tag="qT")
            en_all = enp.tile([HEADS, QMACRO], F16, tag="en")
            for s in range(NSUB):
                xn = xns_cur[s]
                xnT = xnTp.tile([128, 8, 128], F16, tag="xnT")
                ptr = ps_mi.tile([128, 8, 128], F16, tag="mi")
                for t in range(8):
                    nc.tensor.transpose(ptr[:, t, :],
                                        xn[:, t * 128:(t + 1) * 128],
                                        ident[:])
                nc.scalar.copy(xnT[:], ptr[:])
                pq = [ps_half.tile([128, 512], F32, tag="ph", name=f"pq{_i}")
                      for _i in range(2)]
                pq_null = ps_mi.tile([128, HEADS], F32, tag="mi")
                for half in range(2):
                    nc.tensor.matmul(pq[half][:], lhsT=ones_row[0:1, :],
                                     rhs=qbias[0:1, half * 512:(half + 1) * 512],
                                     start=True, stop=False)
                    for kc in range(8):
                        nc.tensor.matmul(
                            pq[half][:],
                            lhsT=xnT[:, kc, :],
                            rhs=wq_sb[:, kc, half * 512:(half + 1) * 512],
                            start=False, stop=(kc == 7))
                nc.tensor.matmul(pq_null[:], lhsT=ones_row[0:1, :],
                                 rhs=qbias_null[0:1, :], start=True, stop=False)
                for kc in range(8):
                    nc.tensor.matmul(pq_null[:], lhsT=xnT[:, kc, :],
                                     rhs=wq_null[:, kc, :],
                                     start=False, stop=(kc == 7))
                qf = qfp.tile([128, DIM], F16, tag="qf")
                rn = _emit_l2norm_heads(nc, small, sq_pool,
                                        qf[:].rearrange("p (h d) -> p h d", d=HD),
                                        [pq[0][:], pq[1][:]], None, tiny_tile)
                null_s = nsp.tile([128, HEADS], F16, tag="ns")
                nc.vector.tensor_mul(null_s[:], pq_null[:], rn[:])
                ptr = ps_mi.tile([128, 8, 128], F16, tag="mi")
                for t in range(8):
                    nc.tensor.transpose(ptr[:, t, :],
                                        qf[:, t * 128:(t + 1) * 128],
                                        ident[:])
                nc.vector.tensor_copy(qT[:, :, s * 128:(s + 1) * 128], ptr[:])
                en_ptr = ps_mi.tile([HEADS, 128], F16, tag="mi")
                nc.tensor.transpose(en_ptr[:], null_s[:], ident[:])
                nc.scalar.activation(out=en_all[:, s * 128:(s + 1) * 128],
                                     in_=en_ptr[:], func=AF.Exp,
                                     bias=expb[0:HEADS, :], scale=SCALE)

            # ---- prefetch + LN for next macro (keeps PE fed at boundary) ----
            if m + 1 < NMACRO:
                xns_cur = issue_x_ln(m + 1)

            # ---- head stage: 8 head pairs ----
            outT = outTp.tile([128, 8, QMACRO], F16, tag="outT")
            for c in range(8):
                h0, h1 = 2 * c, 2 * c + 1
                r_t = rp.tile([1, QMACRO], F32, tag="r")
                rbc = rbcp.tile([128, QMACRO], F32, tag="rbc")
                for par in range(2):  # 0 = even head, 1 = odd head
                    jb = par * HD
                    kT_h = kT[jb:jb + HD, c, :]
                    qT_h = qT[jb:jb + HD, c, :]
                    ets = []
                    for kc in range(2):
                        ps_s = ps_sc.tile([128, QMACRO], F32, tag="sc")
                        nc.tensor.matmul(ps_s[:],
                                         lhsT=kT_h[:, kc * 128:(kc + 1) * 128],
                                         rhs=qT_h, start=True, stop=True)
                        et = etp.tile([128, QMACRO], F16, tag="et")
                        nc.scalar.activation(out=et[:], in_=ps_s[:],
                                             func=AF.Exp, bias=expb[:], scale=SCALE)
                        ets.append(et)
                    if par == 0:
                        po = ps_oe.tile([HD + 1, QMACRO], F32, tag="oe")
                        nc.tensor.matmul(po[:], lhsT=v_e[:, 0, c, :],
                                         rhs=ets[0][:], start=True, stop=False)
                        nc.tensor.matmul(po[:], lhsT=v_e[:, 1, c, :],
                                         rhs=ets[1][:], start=False, stop=False)
                        nc.tensor.matmul(po[:], lhsT=vne_m[:, h0, :],
                                         rhs=en_all[:, :],
                                         start=False, stop=True)
                        r16 = rp.tile([HD + 1, QMACRO], F16, tag="r16")
                        with nc.allow_low_precision(reason="1/sum in f16, 5e-4 rel"):
                            nc.vector.reciprocal(r16[HD:HD + 1, :],
                                                 po[HD:HD + 1, :])
                        pbr = ps_mi.tile([HD, QMACRO], F32, tag="mi")
                        nc.tensor.matmul(pbr[:], lhsT=ones64[HD:HD + 1, :],
                                         rhs=r16[HD:HD + 1, :],
                                         start=True, stop=True)
                        rbf = rbcp.tile([HD, QMACRO], F32, tag="rbf")
                        nc.scalar.copy(rbf[:], pbr[:])
                        nc.vector.tensor_mul(outT[0:HD, c, :], po[0:HD, :],
                                             rbf[:])
                    else:
                        po = ps_oo.tile([128, QMACRO], F32, tag="oo")
                        nc.tensor.matmul(po[:], lhsT=v_o[:, 0, c, :],
                                         rhs=ets[0][:], start=True, stop=False)
                        nc.tensor.matmul(po[:], lhsT=v_o[:, 1, c, :],
                                         rhs=ets[1][:], start=False, stop=False)
                        nc.tensor.matmul(po[:], lhsT=vno_m[:, h1, :],
                                         rhs=en_all[:, :],
                                         start=False, stop=True)
                        nc.vector.reciprocal(r_t[0:1, :], po[0:1, :])
                        nc.gpsimd.partition_broadcast(rbc[:, :], r_t[0:1, :])
                        nc.vector.tensor_mul(outT[64:128, c, :], po[64:128, :],
                                             rbc[64:128, :])

            # ---- output stage: Wo + LN out ----
            for s in range(NSUB):
                r0 = m * QMACRO + s * 128
                pf = [ps_half.tile([128, 512], F32, tag="ph", name=f"pf{_i}")
                      for _i in range(2)]
                for half in range(2):
                    for kc in range(8):
                        nc.tensor.matmul(
                            pf[half][:],
                            lhsT=outT[:, kc, s * 128:(s + 1) * 128],
                            rhs=wo_sb[:, kc, half * 512:(half + 1) * 512],
                            start=(kc == 0), stop=(kc == 7))
                rstd, negmr = _emit_ln_stats(nc, small, [pf[0][:], pf[1][:]],
                                             eps_tile)
                ob16 = obp.tile([128, DIM], F16, tag="ob16")
                for half in range(2):
                    nc.scalar.activation(out=ob16[:, half * 512:(half + 1) * 512],
                                         in_=pf[half][:], func=AF.Identity,
                                         bias=negmr[:], scale=rstd[:])
                obg = obgp.tile([128, DIM], F16, tag="obg")
                nc.vector.tensor_mul(obg[:], ob16[:], log16[:])
                obf = obfp.tile([128, DIM], F32, tag="obf")
                nc.gpsimd.tensor_add(obf[:], obg[:], lob[:])
                nc.sync.dma_start(out=OUT[r0:r0 + 128, :], in_=obf[:])

    nc.compile()
    return nc


_NC_CACHE = None


def kernel(**inputs):
    global _NC_CACHE
    if _NC_CACHE is None:
        _NC_CACHE = build_nc()
    nc = _NC_CACHE

    x = np.asarray(inputs["x"], np.float32)
    context = np.asarray(inputs["context"], np.float32)
    shared = {
        "Wq": np.asarray(inputs["Wq"], np.float32).astype(np.float16),
        "Wkv": np.asarray(inputs["Wkv"], np.float32).astype(np.float16),
        "Wo": np.asarray(inputs["Wo"], np.float32).astype(np.float16),
        "null_kv": np.asarray(inputs["null_kv"], np.float32),
        "q_scale": np.asarray(inputs["q_scale"], np.float32),
        "k_scale": np.asarray(inputs["k_scale"], np.float32),
        "ln_in_g": np.asarray(inputs["ln_in_g"], np.float32),
        "ln_in_b": np.asarray(inputs["ln_in_b"], np.float32),
        "ln_ctx_g": np.asarray(inputs["ln_ctx_g"], np.float32),
        "ln_ctx_b": np.asarray(inputs["ln_ctx_b"], np.float32),
        "ln_out_g": np.asarray(inputs["ln_out_g"], np.float32),
        "ln_out_b": np.asarray(inputs["ln_out_b"], np.float32),
    }
    B, N, _ = x.shape
    in_maps = []
    for c in range(N_CORES):
        b, n0 = c // 2, (c % 2) * ROWS
        in_maps.append({"xs": np.ascontiguousarray(x[b, n0:n0 + ROWS]),
                        "ctx": np.ascontiguousarray(context[b]), **shared})

    res = run_bass_kernel_spmd(nc, in_maps, list(range(N_CORES)))

    out = np.empty((B, N, DIM), np.float32)
    for c in range(N_CORES):
        b, n0 = c // 2, (c % 2) * ROWS
        out[b, n0:n0 + ROWS] = res.results[c]["out"]
    return out


# revision 15
# speedup vs baseline: 1.3134x; 1.1764x over previous
"""Trainium2 Bass kernel for nn_CrossAttention (B=4, N=4096, T=256, DIM=1024,
16 heads x 64 dim, cosine-sim attention with null-kv token, LN in/ctx/out).

Sharding: data-parallel over query rows. Core c handles batch b=c//2, query
rows (c%2)*2048 : (c%2)*2048+2048. The kv projections (tiny: T=256) are
computed redundantly per core; no collectives are needed.

v2 rewrite (cost-model driven):
  - Null-kv scores computed as 16 extra q-projection columns (wq_null =
    Wq'_head-blocks @ (k_null_hat*qs*ks), rank-16) instead of a per-head
    K=1 matmul against a kT null column: kills 64 N=512 PE matmuls and 64
    single-partition Act exps; the null exp becomes one [16, 512] exp/macro.
  - Softmax denominator: reciprocal of the ones-column row of po (PSUM)
    into an SBUF row, partition-broadcast on the idle GPSIMD engine, then
    one aligned DVE multiply. Odd heads get v' column-padded ([0*63, 1, v])
    so their po lands on partitions 63..127 directly: no partition-shift
    DMA, no [64,512] broadcast matmul (pb), no PSUM->SBUF sums copies.
  - rsqrt via exp(-0.5*ln(x)) so every Act function (Identity/Square/Exp/
    Ln/Copy) lives in the natural_log_exp_and_others table: zero
    ACT_TABLE_LOAD switches (LN row-scale precision cancels in the cosine
    attention anyway).
  - All weights (incl. Wkv) prefetched; next macro's x-LayerNorm is issued
    before this macro's head stage so PE never waits on the serial LN chain
    (keeps the Tensor engine p-state ramped at 2.4 GHz).
  - fp16 squares/reduces for the l2 norms (2x/4x DVE modes); ln_out bias-add
    moved to GPSIMD; gain-mul done in fp16.
"""

import numpy as np
from contextlib import ExitStack

import concourse.bass as bass
import concourse.tile as tile
from concourse import bacc, mybir
from concourse.bass_utils import run_bass_kernel_spmd
from concourse.masks import make_identity

F32 = mybir.dt.float32
F16 = mybir.dt.float16
AF = mybir.ActivationFunctionType
AX = mybir.AxisListType
ALU = mybir.AluOpType

DIM = 1024
HEADS = 16
HD = 64
T = 256
SCALE = 8.0
EXPB = -5.545177444479562  # ln(1/256)
LN_EPS = 1e-5
N_CORES = 8
ROWS = 2048
QMACRO = 512
NSUB = QMACRO // 128
NMACRO = ROWS // QMACRO


def _emit_ln_stats(nc, pool_small, in_aps, eps_tile):
    """in_aps: list of [128, 512] APs covering the 1024 row. Returns
    (rstd [128,1], negmr [128,1]) fp32 tiles for (x - m) * rstd.
    rstd = exp(-0.5 * ln(var + eps)) to stay in the exp act table."""
    stats = pool_small.tile([128, 2, 6], F32, tag="lnstats")
    for i, ap in enumerate(in_aps):
        nc.vector.bn_stats(out=stats[:, i, :], in_=ap)
    mv = pool_small.tile([128, 2], F32, tag="lnmv")
    nc.vector.bn_aggr(out=mv[:], in_=stats[:])
    lnv = pool_small.tile([128, 1], F32, tag="lnlnv")
    nc.scalar.activation(out=lnv[:], in_=mv[:, 1:2], func=AF.Ln,
                         bias=eps_tile[:], scale=1.0)
    rstd = pool_small.tile([128, 1], F32, tag="lnrstd")
    nc.scalar.activation(out=rstd[:], in_=lnv[:], func=AF.Exp,
                         bias=0.0, scale=-0.5)
    negmr = pool_small.tile([128, 1], F32, tag="lnnegmr")
    nc.vector.scalar_tensor_tensor(out=negmr[:], in0=mv[:, 0:1], scalar=-1.0,
                                   in1=rstd[:], op0=ALU.mult, op1=ALU.mult)
    return rstd, negmr


def _emit_l2norm_heads(nc, pool_small, sq_pool, out_ap3, in_half_aps, scale_tile, tiny):
    """in_half_aps: [pa, pb] fp32 psum APs [128, 512] (heads 0-7, 8-15).
    out_ap3: [128, 16, 64] fp16 sbuf AP. out = in * rsqrt(ssq_head)
    (* scale_tile [128,64] if given). Returns rn [128, 16] fp32."""
    sq = sq_pool.tile([128, 1024], F16, tag="sq")
    nc.scalar.activation(out=sq[:, 0:512], in_=in_half_aps[0], func=AF.Square,
                         bias=0.0, scale=1.0)
    nc.scalar.activation(out=sq[:, 512:1024], in_=in_half_aps[1], func=AF.Square,
                         bias=0.0, scale=1.0)
    ssq = pool_small.tile([128, 16], F16, tag="ssq")
    with nc.allow_low_precision(reason="l2norm ssq in fp16; rel err ~1e-3 ok"):
        nc.vector.reduce_sum(out=ssq[:], in_=sq[:].rearrange("p (h d) -> p h d", d=HD),
                             axis=AX.X)
    lns = pool_small.tile([128, 16], F32, tag="l2ln")
    nc.scalar.activation(out=lns[:], in_=ssq[:], func=AF.Ln,
                         bias=tiny[:], scale=1.0)
    rn = pool_small.tile([128, 16], F32, tag="l2rn")
    nc.scalar.activation(out=rn[:], in_=lns[:], func=AF.Exp,
                         bias=0.0, scale=-0.5)
    for i in range(2):
        h0 = i * 8
        out_h = out_ap3[:, h0:h0 + 8, :]
        in3 = in_half_aps[i].rearrange("p (h d) -> p h d", d=HD)
        nc.vector.tensor_mul(
            out_h, in3,
            rn[:, h0:h0 + 8].unsqueeze(-1).broadcast_to([128, 8, HD]))
        if scale_tile is not None:
            nc.vector.tensor_mul(
                out_h, out_h,
                scale_tile[:].unsqueeze(1).broadcast_to([128, 8, HD]))
    return rn


def _load_bcast(nc, dst_tile, dram_ap, parts=128):
    ap = bass.AP(tensor=dram_ap.tensor, offset=dram_ap.offset,
                 ap=[[0, parts]] + dram_ap.ap)
    nc.sync.dma_start(out=dst_tile[:parts, :], in_=ap)


def _load_cols(nc, dst_tile, dram_ap):
    """Load a [1024] dram vector as [128, 8] (dst[p, kc] = v[kc*128+p])."""
    ap = bass.AP(tensor=dram_ap.tensor, offset=dram_ap.offset,
                 ap=[[1, 128], [128, 8]])
    nc.sync.dma_start(out=dst_tile[:, :], in_=ap)


def build_nc():
    nc = bacc.Bacc("TRN2", debug=False)

    XS = nc.dram_tensor("xs", [ROWS, DIM], F32, kind="ExternalInput")
    CTX = nc.dram_tensor("ctx", [T, DIM], F32, kind="ExternalInput")
    WQ = nc.dram_tensor("Wq", [DIM, DIM], F16, kind="ExternalInput")
    WKV = nc.dram_tensor("Wkv", [DIM, 2 * DIM], F16, kind="ExternalInput")
    WO = nc.dram_tensor("Wo", [DIM, DIM], F16, kind="ExternalInput")
    NKV = nc.dram_tensor("null_kv", [2, HD], F32, kind="ExternalInput")
    QS = nc.dram_tensor("q_scale", [HD], F32, kind="ExternalInput")
    KS = nc.dram_tensor("k_scale", [HD], F32, kind="ExternalInput")
    LIG = nc.dram_tensor("ln_in_g", [DIM], F32, kind="ExternalInput")
    LIB = nc.dram_tensor("ln_in_b", [DIM], F32, kind="ExternalInput")
    LCG = nc.dram_tensor("ln_ctx_g", [DIM], F32, kind="ExternalInput")
    LCB = nc.dram_tensor("ln_ctx_b", [DIM], F32, kind="ExternalInput")
    LOG = nc.dram_tensor("ln_out_g", [DIM], F32, kind="ExternalInput")
    LOB = nc.dram_tensor("ln_out_b", [DIM], F32, kind="ExternalInput")
    OUT = nc.dram_tensor("out", [ROWS, DIM], F32, kind="ExternalOutput")

    with tile.TileContext(nc) as tc, ExitStack() as ctx:
        consts = ctx.enter_context(tc.tile_pool(name="consts", bufs=1))
        weights = ctx.enter_context(tc.tile_pool(name="weights", bufs=1))
        kvpool = ctx.enter_context(tc.tile_pool(name="kvpool", bufs=1))
        small = ctx.enter_context(tc.tile_pool(name="small", bufs=3))
        sq_pool = ctx.enter_context(tc.tile_pool(name="sqp", bufs=2))

        # PSUM: half(2) + mi(2) + sc(2) + oe(1) + oo(1) = 8 banks
        ps_half = ctx.enter_context(tc.tile_pool(name="ps_half", bufs=2, space="PSUM"))
        ps_mi = ctx.enter_context(tc.tile_pool(name="ps_mi", bufs=2, space="PSUM"))
        ps_sc = ctx.enter_context(tc.tile_pool(name="ps_sc", bufs=2, space="PSUM"))
        ps_oe = ctx.enter_context(tc.tile_pool(name="ps_oe", bufs=1, space="PSUM"))
        ps_oo = ctx.enter_context(tc.tile_pool(name="ps_oo", bufs=1, space="PSUM"))

        # ---------------- input DMAs, highest priority first ----------------
        nkvb = consts.tile([128, 2, HD], F32)
        _load_bcast(nc, nkvb.rearrange("p a b -> p (a b)"),
                    NKV[:, :].rearrange("a b -> (a b)"))
        lcg = kvpool.tile([128, DIM], F32)
        lcb = kvpool.tile([128, DIM], F32)
        _load_bcast(nc, lcg, LCG[:])
        _load_bcast(nc, lcb, LCB[:])
        qsc = consts.tile([128, HD], F32)
        _load_bcast(nc, qsc, QS[:])
        ksc = consts.tile([128, HD], F32)
        _load_bcast(nc, ksc, KS[:])

        xin = ctx.enter_context(tc.tile_pool(name="xin", bufs=6))
        xnp = ctx.enter_context(tc.tile_pool(name="xnp", bufs=8))
        wkv_ctx = ExitStack()
        wkvp = wkv_ctx.enter_context(tc.tile_pool(name="wkvp", bufs=1))
        wkv_sb = wkvp.tile([128, 8, 2 * DIM], F16)
        for kc in range(8):
            nc.sync.dma_start(out=wkv_sb[:, kc, :],
                              in_=WKV[kc * 128:(kc + 1) * 128, :])
        ctx_t = []
        for i in range(2):
            ct = kvpool.tile([128, DIM], F32, tag=f"ctx{i}", name=f"ctx{i}")
            nc.sync.dma_start(out=ct[:], in_=CTX[i * 128:(i + 1) * 128, :])
            ctx_t.append(ct)

        x0_tiles = []
        for s in range(NSUB):
            x_t = xin.tile([128, DIM], F32, tag="x")
            nc.sync.dma_start(out=x_t[:], in_=XS[s * 128:(s + 1) * 128, :])
            x0_tiles.append(x_t)

        gcol = consts.tile([128, 8], F32)
        _load_cols(nc, gcol, LIG[:])
        bcol = consts.tile([128, 8], F32)
        _load_cols(nc, bcol, LIB[:])

        wq_sb = weights.tile([128, 8, DIM], F16)
        for kc in range(8):
            nc.sync.dma_start(out=wq_sb[:, kc, :], in_=WQ[kc * 128:(kc + 1) * 128, :])

        log32 = consts.tile([128, DIM], F32)
        _load_bcast(nc, log32, LOG[:])
        lob = consts.tile([128, DIM], F32)
        _load_bcast(nc, lob, LOB[:])
        log16 = consts.tile([128, DIM], F16)
        nc.vector.tensor_copy(log16[:], log32[:])

        wo_sb = weights.tile([128, 8, DIM], F16)
        for kc in range(8):
            nc.sync.dma_start(out=wo_sb[:, kc, :], in_=WO[kc * 128:(kc + 1) * 128, :])

        # ---------------- small consts ----------------
        ident = consts.tile([128, 128], F16)
        make_identity(nc, ident)
        # one act-table for the whole kernel (Exp/Ln/Identity/Square/Copy all
        # live in natural_log_exp_and_others) - pre-loading it stops
        # insert_act_table_loads from thrashing between per-func defaults
        from concourse.hw_specs import get_activation_tables
        _tabs = get_activation_tables(nc.m.arch)
        _nle_idx = list(_tabs).index("natural_log_exp_and_others")
        nc.scalar.add_instruction(mybir.InstLoadActFuncSet(
            name=f"I-{nc.next_id()}", ins=[], outs=[],
            act_func_set_id=_nle_idx))
        eps_tile = consts.tile([128, 1], F32)
        nc.vector.memset(eps_tile[:], LN_EPS)
        tiny_tile = consts.tile([128, 1], F32)
        nc.vector.memset(tiny_tile[:], 1e-12)
        onesf = consts.tile([128, 1], F32)
        nc.vector.memset(onesf[:], 1.0)
        expb = consts.tile([128, 1], F32)
        nc.vector.memset(expb[:], EXPB)
        ones64 = consts.tile([128, HD], F16)
        nc.vector.memset(ones64[:], 1.0)
        ones_row = consts.tile([1, 128], F16)
        nc.vector.tensor_copy(ones_row[0:1, :], onesf[0:1, 0:1].broadcast_to([1, 128]))
        kqsc = consts.tile([128, HD], F32)
        nc.vector.tensor_mul(kqsc[:], ksc[:], qsc[:])
        bcol16 = consts.tile([128, 8], F16)
        nc.vector.tensor_copy(bcol16[:], bcol[:])

        # ---------------- null-kv prep (all 128 partitions in parallel) -----
        # w = k_null_hat * (qs*ks)  [128, 64] f32 (same value on every row)
        sqn = consts.tile([128, HD], F32)
        nc.vector.tensor_mul(sqn[:], nkvb[:, 0, :], nkvb[:, 0, :])
        ssqn = consts.tile([128, 1], F32)
        nc.vector.reduce_sum(out=ssqn[:], in_=sqn[:], axis=AX.X)
        lnn = consts.tile([128, 1], F32)
        nc.scalar.activation(out=lnn[:], in_=ssqn[:], func=AF.Ln,
                             bias=tiny_tile[:], scale=1.0)
        rnn = consts.tile([128, 1], F32)
        nc.scalar.activation(out=rnn[:], in_=lnn[:], func=AF.Exp,
                             bias=0.0, scale=-0.5)
        w_b = consts.tile([128, HD], F16)
        nc.vector.scalar_tensor_tensor(out=w_b[:], in0=nkvb[:, 0, :], scalar=rnn[:],
                                       in1=kqsc[:], op0=ALU.mult, op1=ALU.mult)
        # per-head masked null-v weights (K=16, base partition 0):
        # vne_m[p, h, :] = (p==h) ? [v_null, 1] : 0   [16, 16, 65]
        # vno_m[p, h, :] = (p==h) ? [1, 0*63, v_null] : 0 [16, 16, 128]
        # (odd-head sums row sits at partition 0: partition_broadcast only
        #  works base-0 -> base-0 on hardware)
        vrow_e = consts.tile([128, HD + 1], F16)
        nc.vector.tensor_copy(vrow_e[:, 0:HD], nkvb[:, 1, :])
        nc.vector.tensor_copy(vrow_e[:, HD:HD + 1], onesf[:, 0:1])
        vrow_o = consts.tile([128, 128], F16)
        nc.vector.memset(vrow_o[:, 0:64], 0.0)
        nc.vector.tensor_copy(vrow_o[:, 0:1], onesf[:, 0:1])
        nc.vector.tensor_copy(vrow_o[:, 64:128], nkvb[:, 1, :])
        vne_m = kvpool.tile([16, HEADS, HD + 1], F16)
        nc.vector.tensor_mul(
            vne_m[:],
            ident[0:16, 0:16].unsqueeze(-1).broadcast_to([16, HEADS, HD + 1]),
            vrow_e[0:16, :].unsqueeze(1).broadcast_to([16, HEADS, HD + 1]))
        vno_m = kvpool.tile([16, HEADS, 128], F16)
        nc.vector.tensor_mul(
            vno_m[:],
            ident[0:16, 0:16].unsqueeze(-1).broadcast_to([16, HEADS, 128]),
            vrow_o[0:16, :].unsqueeze(1).broadcast_to([16, HEADS, 128]))

        # ---------------- phase K: context -> kT, v_e, v_o ----------------
        kT = kvpool.tile([128, 8, T], F16)
        v_e = kvpool.tile([128, 2, 8, HD + 1], F16)
        nc.vector.tensor_copy(
            v_e[:, :, :, HD:HD + 1],
            onesf[:, 0:1].unsqueeze(1).unsqueeze(1).broadcast_to([128, 2, 8, 1]))
        v_o = kvpool.tile([128, 2, 8, 128], F16)
        nc.vector.memset(v_o[:, :, :, 0:64], 0.0)
        nc.vector.tensor_copy(
            v_o[:, :, :, 0:1],
            onesf[:, 0:1].unsqueeze(1).unsqueeze(1).broadcast_to([128, 2, 8, 1]))

        with ExitStack() as kctx:
            pk = kctx.enter_context(tc.tile_pool(name="pk", bufs=2))
            pk1 = kctx.enter_context(tc.tile_pool(name="pk1", bufs=1))

            cnT = pk1.tile([128, 8, T], F16)
            for i in range(2):
                rstd, negmr = _emit_ln_stats(
                    nc, small, [ctx_t[i][:, 0:512], ctx_t[i][:, 512:1024]], eps_tile)
                cnf = pk.tile([128, DIM], F32, tag="cnf")
                nc.scalar.activation(out=cnf[:], in_=ctx_t[i][:], func=AF.Identity,
                                     bias=negmr[:], scale=rstd[:])
                nc.vector.tensor_mul(cnf[:], cnf[:], lcg[:])
                cn = pk.tile([128, DIM], F16, tag="cn")
                nc.vector.tensor_add(cn[:], cnf[:], lcb[:])
                ptr = ps_mi.tile([128, 8, 128], F16, tag="mi")
                for t in range(8):
                    nc.tensor.transpose(ptr[:, t, :],
                                        cn[:, t * 128:(t + 1) * 128],
                                        ident[:])
                nc.vector.tensor_copy(cnT[:, :, i * 128:(i + 1) * 128], ptr[:])

            for i in range(2):
                for which in (0, 1):  # 0 = k, 1 = v
                    ph = [ps_half.tile([128, 512], F32, tag="ph", name=f"ph{_i}")
                          for _i in range(2)]
                    for half in range(2):
                        for kc in range(8):
                            nc.tensor.matmul(
                                ph[half][:],
                                lhsT=cnT[:, kc, i * 128:(i + 1) * 128],
                                rhs=wkv_sb[:, kc,
                                           which * DIM + half * 512:
                                           which * DIM + (half + 1) * 512],
                                start=(kc == 0), stop=(kc == 7))
                    if which == 0:
                        kfin = pk.tile([128, DIM], F16, tag="kfin")
                        _emit_l2norm_heads(
                            nc, small, sq_pool,
                            kfin[:].rearrange("p (h d) -> p h d", d=HD),
                            [ph[0][:], ph[1][:]], kqsc, tiny_tile)
                        ptr = ps_mi.tile([128, 8, 128], F16, tag="mi")
                        for t in range(8):
                            nc.tensor.transpose(
                                ptr[:, t, :],
                                kfin[:, t * 128:(t + 1) * 128],
                                ident[:])
                        nc.vector.tensor_copy(
                            kT[:, :, i * 128:(i + 1) * 128], ptr[:])
                    else:
                        for half in range(2):
                            ph3 = ph[half][:].rearrange("p (h d) -> p h d", d=HD)
                            for hh in range(8):
                                h = half * 8 + hh
                                c = h // 2
                                if h % 2 == 0:
                                    nc.vector.tensor_copy(
                                        v_e[:, i, c, 0:HD], ph3[:, hh, :])
                                else:
                                    nc.vector.tensor_copy(
                                        v_o[:, i, c, 64:128], ph3[:, hh, :])

            # qbias = ln_in_b @ Wq (rank-1), before Wq is gain-scaled
            qbias = kvpool.tile([1, DIM], F16)
            for half in range(2):
                pqb = ps_mi.tile([1, 512], F32, tag="mi")
                for kc in range(8):
                    nc.tensor.matmul(pqb[:], lhsT=bcol16[:, kc:kc + 1],
                                     rhs=wq_sb[:, kc, half * 512:(half + 1) * 512],
                                     start=(kc == 0), stop=(kc == 7))
                nc.vector.tensor_copy(qbias[0:1, half * 512:(half + 1) * 512], pqb[:])
            # Wq' = diag(ln_in_g) @ Wq
            for kc in range(8):
                nc.vector.tensor_scalar_mul(wq_sb[:, kc, :], wq_sb[:, kc, :],
                                            gcol[:, kc:kc + 1])
            # wq_null[:, kc, h] = sum_e Wq'[:, kc, h*64+e] * w[e]
            wq_null = weights.tile([128, 8, HEADS], F16)
            for kc in range(8):
                tmp = sq_pool.tile([128, 1024], F16, tag="sq")
                nc.vector.tensor_mul(
                    tmp[:].rearrange("p (h d) -> p h d", d=HD),
                    wq_sb[:, kc, :].rearrange("p (h d) -> p h d", d=HD),
                    w_b[:].unsqueeze(1).broadcast_to([128, HEADS, HD]))
                with nc.allow_low_precision(reason="rank-16 null cols fp16"):
                    nc.vector.reduce_sum(
                        out=wq_null[:, kc, :],
                        in_=tmp[:].rearrange("p (h d) -> p h d", d=HD), axis=AX.X)
            # qbias_null[h] = sum_e qbias[h*64+e] * w[e]
            qbn_t = pk1.tile([1, DIM], F16)
            nc.vector.tensor_mul(
                qbn_t[0:1, :].rearrange("p (h d) -> p h d", d=HD),
                qbias[0:1, :].rearrange("p (h d) -> p h d", d=HD),
                w_b[0:1, :].unsqueeze(1).broadcast_to([1, HEADS, HD]))
            qbias_null = kvpool.tile([1, HEADS], F16)
            with nc.allow_low_precision(reason="null bias fp16"):
                nc.vector.reduce_sum(
                    out=qbias_null[0:1, :],
                    in_=qbn_t[0:1, :].rearrange("p (h d) -> p h d", d=HD), axis=AX.X)
        wkv_ctx.close()

        # ---------------- main loop pools ----------------
        xnTp = ctx.enter_context(tc.tile_pool(name="xnTp", bufs=2))
        qfp = ctx.enter_context(tc.tile_pool(name="qfp", bufs=2))
        qTp = ctx.enter_context(tc.tile_pool(name="qTp", bufs=2))
        etp = ctx.enter_context(tc.tile_pool(name="etp", bufs=4))
        rp = ctx.enter_context(tc.tile_pool(name="rp", bufs=2))
        rbcp = ctx.enter_context(tc.tile_pool(name="rbcp", bufs=2))
        enp = ctx.enter_context(tc.tile_pool(name="enp", bufs=2))
        nsp = ctx.enter_context(tc.tile_pool(name="nsp", bufs=4))
        outTp = ctx.enter_context(tc.tile_pool(name="outTp", bufs=2))
        obp = ctx.enter_context(tc.tile_pool(name="obp", bufs=2))
        obgp = ctx.enter_context(tc.tile_pool(name="obgp", bufs=2))
        obfp = ctx.enter_context(tc.tile_pool(name="obfp", bufs=2))

        def issue_x_ln(m, x_tiles=None):
            """DMA + LayerNorm for macro m's 4 subtiles. Returns xn tiles."""
            xns = []
            for s in range(NSUB):
                r0 = m * QMACRO + s * 128
                if x_tiles is not None:
                    x_t = x_tiles[s]
                else:
                    x_t = xin.tile([128, DIM], F32, tag="x")
                    nc.sync.dma_start(out=x_t[:], in_=XS[r0:r0 + 128, :])
                rstd, negmr = _emit_ln_stats(
                    nc, small, [x_t[:, 0:512], x_t[:, 512:1024]], eps_tile)
                xn = xnp.tile([128, DIM], F16, tag="xn")
                nc.scalar.activation(out=xn[:], in_=x_t[:], func=AF.Identity,
                                     bias=negmr[:], scale=rstd[:])
                xns.append(xn)
            return xns

        xns_cur = issue_x_ln(0, x0_tiles)

        for m in range(NMACRO):
            # ---- B phase: transposes + q projection + l2norm + null cols ----
            qT = qTp.tile([128, 8, QMACRO], F16, tag="qT")
            en_all = enp.tile([HEADS, QMACRO], F16, tag="en")
            for s in range(NSUB):
                xn = xns_cur[s]
                xnT = xnTp.tile([128, 8, 128], F16, tag="xnT")
                ptr = ps_mi.tile([128, 8, 128], F16, tag="mi")
                for t in range(8):
                    nc.tensor.transpose(ptr[:, t, :],
                                        xn[:, t * 128:(t + 1) * 128],
                                        ident[:])
                nc.scalar.copy(xnT[:], ptr[:])
                pq = [ps_half.tile([128, 512], F32, tag="ph", name=f"pq{_i}")
                      for _i in range(2)]
                pq_null = ps_mi.tile([128, HEADS], F32, tag="mi")
                for half in range(2):
                    nc.tensor.matmul(pq[half][:], lhsT=ones_row[0:1, :],
                                     rhs=qbias[0:1, half * 512:(half + 1) * 512],
                                     start=True, stop=False)
                    for kc in range(8):
                        nc.tensor.matmul(
                            pq[half][:],
                            lhsT=xnT[:, kc, :],
                            rhs=wq_sb[:, kc, half * 512:(half + 1) * 512],
                            start=False, stop=(kc == 7))
                nc.tensor.matmul(pq_null[:], lhsT=ones_row[0:1, :],
                                 rhs=qbias_null[0:1, :], start=True, stop=False)
                for kc in range(8):
                    nc.tensor.matmul(pq_null[:], lhsT=xnT[:, kc, :],
                                     rhs=wq_null[:, kc, :],
                                     start=False, stop=(kc == 7))
                qf = qfp.tile([128, DIM], F16, tag="qf")
                rn = _emit_l2norm_heads(nc, small, sq_pool,
                                        qf[:].rearrange("p (h d) -> p h d", d=HD),
                                        [pq[0][:], pq[1][:]], None, tiny_tile)
                null_s = nsp.tile([128, HEADS], F16, tag="ns")
                nc.vector.tensor_mul(null_s[:], pq_null[:], rn[:])
                ptr = ps_mi.tile([128, 8, 128], F16, tag="mi")
                for t in range(8):
                    nc.tensor.transpose(ptr[:, t, :],
                                        qf[:, t * 128:(t + 1) * 128],
                                        ident[:])
                nc.vector.tensor_copy(qT[:, :, s * 128:(s + 1) * 128], ptr[:])
                en_ptr = ps_mi.tile([HEADS, 128], F16, tag="mi")
                nc.tensor.transpose(en_ptr[:], null_s[:], ident[:])
                nc.scalar.activation(out=en_all[:, s * 128:(s + 1) * 128],
                                     in_=en_ptr[:], func=AF.Exp,
                                     bias=expb[0:HEADS, :], scale=SCALE)

            # ---- prefetch + LN for next macro (keeps PE fed at boundary) ----
            if m + 1 < NMACRO:
                xns_cur = issue_x_ln(m + 1)

            # ---- head stage: 8 head pairs ----
            outT = outTp.tile([128, 8, QMACRO], F16, tag="outT")
            for c in range(8):
                h0, h1 = 2 * c, 2 * c + 1
                r_t = rp.tile([1, QMACRO], F32, tag="r")
                rbc = rbcp.tile([128, QMACRO], F32, tag="rbc")
                for par in range(2):  # 0 = even head, 1 = odd head
                    jb = par * HD
                    kT_h = kT[jb:jb + HD, c, :]
                    qT_h = qT[jb:jb + HD, c, :]
                    ets = []
                    for kc in range(2):
                        ps_s = ps_sc.tile([128, QMACRO], F32, tag="sc")
                        nc.tensor.matmul(ps_s[:],
                                         lhsT=kT_h[:, kc * 128:(kc + 1) * 128],
                                         rhs=qT_h, start=True, stop=True)
                        et = etp.tile([128, QMACRO], F16, tag="et")
                        nc.scalar.activation(out=et[:], in_=ps_s[:],
                                             func=AF.Exp, bias=expb[:], scale=SCALE)
                        ets.append(et)
                    if par == 0:
                        po = ps_oe.tile([HD + 1, QMACRO], F32, tag="oe")
                        nc.tensor.matmul(po[:], lhsT=v_e[:, 0, c, :],
                                         rhs=ets[0][:], start=True, stop=False)
                        nc.tensor.matmul(po[:], lhsT=v_e[:, 1, c, :],
                                         rhs=ets[1][:], start=False, stop=False)
                        nc.tensor.matmul(po[:], lhsT=vne_m[:, h0, :],
                                         rhs=en_all[:, :],
                                         start=False, stop=True)
                        r16 = rp.tile([HD + 1, QMACRO], F16, tag="r16")
                        with nc.allow_low_precision(reason="1/sum in f16, 5e-4 rel"):
                            nc.vector.reciprocal(r16[HD:HD + 1, :],
                                                 po[HD:HD + 1, :])
                        pbr = ps_mi.tile([HD, QMACRO], F32, tag="mi")
                        nc.tensor.matmul(pbr[:], lhsT=ones64[HD:HD + 1, :],
                                         rhs=r16[HD:HD + 1, :],
                                         start=True, stop=True)
                        rbf = rbcp.tile([HD, QMACRO], F32, tag="rbf")
                        nc.scalar.copy(rbf[:], pbr[:])
                        nc.vector.tensor_mul(outT[0:HD, c, :], po[0:HD, :],
                                             rbf[:])
                    else:
                        po = ps_oo.tile([128, QMACRO], F32, tag="oo")
                        nc.tensor.matmul(po[:], lhsT=v_o[:, 0, c, :],
                                         rhs=ets[0][:], start=True, stop=False)
                        nc.tensor.matmul(po[:], lhsT=v_o[:, 1, c, :],
                                         rhs=ets[1][:], start=False, stop=False)
                        nc.tensor.matmul(po[:], lhsT=vno_m[:, h1, :],
                                         rhs=en_all[:, :],
                                         start=False, stop=True)
                        nc.vector.reciprocal(r_t[0:1, :], po[0:1, :])
                        nc.gpsimd.partition_broadcast(rbc[:, :], r_t[0:1, :])
                        nc.vector.tensor_mul(outT[64:128, c, :], po[64:128, :],
                                             rbc[64:128, :])

            # ---- output stage: Wo + LN out ----
            for s in range(NSUB):
                r0 = m * QMACRO + s * 128
                pf = [ps_half.tile([128, 512], F32, tag="ph", name=f"pf{_i}")
                      for _i in range(2)]
                for half in range(2):
                    for kc in range(8):
                        nc.tensor.matmul(
                            pf[half][:],
                            lhsT=outT[:, kc, s * 128:(s + 1) * 128],
                            rhs=wo_sb[:, kc, half * 512:(half + 1) * 512],
                            start=(kc == 0), stop=(kc == 7))
                rstd, negmr = _emit_ln_stats(nc, small, [pf[0][:], pf[1][:]],
                                             eps_tile)
                ob16 = obp.tile([128, DIM], F16, tag="ob16")
                for half in range(2):
                    nc.scalar.activation(out=ob16[:, half * 512:(half + 1) * 512],
                                         in_=pf[half][:], func=AF.Identity,
                                         bias=negmr[:], scale=rstd[:])
                obg = obgp.tile([128, DIM], F16, tag="obg")
                nc.vector.tensor_mul(obg[:], ob16[:], log16[:])
                obf = obfp.tile([128, DIM], F32, tag="obf")
                nc.gpsimd.tensor_add(obf[:], obg[:], lob[:])
                nc.sync.dma_start(out=OUT[r0:r0 + 128, :], in_=obf[:])

    nc.compile()
    return nc


_NC_CACHE = None


def kernel(**inputs):
    global _NC_CACHE
    if _NC_CACHE is None:
        _NC_CACHE = build_nc()
    nc = _NC_CACHE

    x = np.asarray(inputs["x"], np.float32)
    context = np.asarray(inputs["context"], np.float32)
    shared = {
        "Wq": np.asarray(inputs["Wq"], np.float32).astype(np.float16),
        "Wkv": np.asarray(inputs["Wkv"], np.float32).astype(np.float16),
        "Wo": np.asarray(inputs["Wo"], np.float32).astype(np.float16),
        "null_kv": np.asarray(inputs["null_kv"], np.float32),
        "q_scale": np.asarray(inputs["q_scale"], np.float32),
        "k_scale": np.asarray(inputs["k_scale"], np.float32),
        "ln_in_g": np.asarray(inputs["ln_in_g"], np.float32),
        "ln_in_b": np.asarray(inputs["ln_in_b"], np.float32),
        "ln_ctx_g": np.asarray(inputs["ln_ctx_g"], np.float32),
        "ln_ctx_b": np.asarray(inputs["ln_ctx_b"], np.float32),
        "ln_out_g": np.asarray(inputs["ln_out_g"], np.float32),
        "ln_out_b": np.asarray(inputs["ln_out_b"], np.float32),
    }
    B, N, _ = x.shape
    in_maps = []
    for c in range(N_CORES):
        b, n0 = c // 2, (c % 2) * ROWS
        in_maps.append({"xs": np.ascontiguousarray(x[b, n0:n0 + ROWS]),
                        "ctx": np.ascontiguousarray(context[b]), **shared})

    res = run_bass_kernel_spmd(nc, in_maps, list(range(N_CORES)))

    out = np.empty((B, N, DIM), np.float32)
    for c in range(N_CORES):
        b, n0 = c // 2, (c % 2) * ROWS
        out[b, n0:n0 + ROWS] = res.results[c]["out"]
    return out


# revision 16
# speedup vs baseline: 1.8050x; 1.3744x over previous
"""Trainium2 Bass kernel for nn_CrossAttention (B=4, N=4096, T=256, DIM=1024,
16 heads x 64 dim, cosine-sim attention with null-kv token, LN in/ctx/out).

Sharding: data-parallel over query rows. Core c handles batch b=c//2, query
rows (c%2)*2048 : (c%2)*2048+2048. The kv projections (tiny: T=256) are
computed redundantly per core; no collectives are needed.

Device program (cost-model driven):
  - Host folds ln_in/ln_ctx gain+bias into Wq/Wkv (+rank-1 bias rows) and
    precomputes the null-kv artifacts: wq_null = Wq'_head-blocks @
    (k_null_hat*qs*ks) adds 16 columns to the q projection that produce the
    null scores, so no per-head K=1 score matmul and no per-head 1-row exp.
  - Softmax denominator rides a ones-column of v'. Odd heads' v' is padded
    [1, 0*63, v] so their attention output lands on PSUM partitions 64..127
    directly (no partition-shift DMA); their 1/sum row sits at partition 0
    and is partition-broadcast on the idle GPSIMD engine. Even heads
    broadcast 1/sum with a K=1 matmul (base-partition rules allow no
    cheaper path).
  - rsqrt = exp(-0.5*ln(x)) keeps every Act function in the single
    natural_log_exp_and_others table (explicitly pre-loaded): one
    ACT_TABLE_LOAD total. LN row-scale precision cancels in the cosine
    attention, out-LN sees ~1e-3 which is inside budget.
  - Loop order per macro: LN(m+1) -> heads(m) -> B(m+1) -> Wo+LN-out(m),
    with the q-side transposes software-pipelined one subtile behind the
    projection matmuls, so the Tensor engine never waits on the serial
    LN / l2norm chains and stays p-state ramped.
"""

import numpy as np
from contextlib import ExitStack

import concourse.bass as bass
import concourse.tile as tile
from concourse import bacc, mybir
from concourse.bass_utils import run_bass_kernel_spmd
from concourse.masks import make_identity

F32 = mybir.dt.float32
F16 = mybir.dt.float16
AF = mybir.ActivationFunctionType
AX = mybir.AxisListType
ALU = mybir.AluOpType

DIM = 1024
HEADS = 16
HD = 64
T = 256
SCALE = 8.0
EXPB = -5.545177444479562  # ln(1/256)
LN_EPS = 1e-5
NORM_EPS = 1e-12
N_CORES = 8
ROWS = 2048
QMACRO = 512
NSUB = QMACRO // 128
NMACRO = ROWS // QMACRO


def _emit_ln_stats(nc, pool_small, in_aps, eps_tile):
    """Returns (rstd [128,1], negmr [128,1]) fp32 for (x - m) * rstd.
    rstd = exp(-0.5 * ln(var + eps)) to stay in the exp act table."""
    stats = pool_small.tile([128, 2, 6], F32, tag="lnstats")
    for i, ap in enumerate(in_aps):
        nc.vector.bn_stats(out=stats[:, i, :], in_=ap)
    mv = pool_small.tile([128, 2], F32, tag="lnmv")
    nc.vector.bn_aggr(out=mv[:], in_=stats[:])
    lnv = pool_small.tile([128, 1], F32, tag="lnlnv")
    nc.scalar.activation(out=lnv[:], in_=mv[:, 1:2], func=AF.Ln,
                         bias=eps_tile[:], scale=1.0)
    rstd = pool_small.tile([128, 1], F32, tag="lnrstd")
    nc.scalar.activation(out=rstd[:], in_=lnv[:], func=AF.Exp,
                         bias=0.0, scale=-0.5)
    negmr = pool_small.tile([128, 1], F32, tag="lnnegmr")
    nc.vector.scalar_tensor_tensor(out=negmr[:], in0=mv[:, 0:1], scalar=-1.0,
                                   in1=rstd[:], op0=ALU.mult, op1=ALU.mult)
    return rstd, negmr


def _emit_l2norm_heads(nc, pool_small, sq_pool, out_ap3, in_half_aps,
                       scale_tile, tiny):
    """out = in * rsqrt(ssq_head) (* scale_tile). Returns rn [128,16] f32."""
    sq = sq_pool.tile([128, 1024], F16, tag="sq")
    nc.scalar.activation(out=sq[:, 0:512], in_=in_half_aps[0], func=AF.Square,
                         bias=0.0, scale=1.0)
    nc.scalar.activation(out=sq[:, 512:1024], in_=in_half_aps[1], func=AF.Square,
                         bias=0.0, scale=1.0)
    ssq = pool_small.tile([128, 16], F16, tag="ssq")
    with nc.allow_low_precision(reason="l2norm ssq in fp16; ~1e-3 rel ok"):
        nc.vector.reduce_sum(out=ssq[:],
                             in_=sq[:].rearrange("p (h d) -> p h d", d=HD),
                             axis=AX.X)
    lns = pool_small.tile([128, 16], F32, tag="l2ln")
    nc.scalar.activation(out=lns[:], in_=ssq[:], func=AF.Ln,
                         bias=tiny[:], scale=1.0)
    rn = pool_small.tile([128, 16], F32, tag="l2rn")
    nc.scalar.activation(out=rn[:], in_=lns[:], func=AF.Exp,
                         bias=0.0, scale=-0.5)
    for i in range(2):
        h0 = i * 8
        out_h = out_ap3[:, h0:h0 + 8, :]
        in3 = in_half_aps[i].rearrange("p (h d) -> p h d", d=HD)
        nc.vector.tensor_mul(
            out_h, in3,
            rn[:, h0:h0 + 8].unsqueeze(-1).broadcast_to([128, 8, HD]))
        if scale_tile is not None:
            nc.vector.tensor_mul(
                out_h, out_h,
                scale_tile[:].unsqueeze(1).broadcast_to([128, 8, HD]))
    return rn


def _load_bcast(nc, dst_tile, dram_ap, parts=128):
    ap = bass.AP(tensor=dram_ap.tensor, offset=dram_ap.offset,
                 ap=[[0, parts]] + dram_ap.ap)
    nc.sync.dma_start(out=dst_tile[:parts, :], in_=ap)


def build_nc():
    nc = bacc.Bacc("TRN2", debug=False)

    XS = nc.dram_tensor("xs", [ROWS, DIM], F32, kind="ExternalInput")
    CTX = nc.dram_tensor("ctx", [T, DIM], F32, kind="ExternalInput")
    WQ = nc.dram_tensor("Wq", [DIM, DIM], F16, kind="ExternalInput")
    WKV = nc.dram_tensor("Wkv", [DIM, 2 * DIM], F16, kind="ExternalInput")
    WO = nc.dram_tensor("Wo", [DIM, DIM], F16, kind="ExternalInput")
    QBIAS = nc.dram_tensor("qbias", [1, DIM], F16, kind="ExternalInput")
    KVBIAS = nc.dram_tensor("kvbias", [1, 2 * DIM], F16, kind="ExternalInput")
    WQN = nc.dram_tensor("wq_null", [DIM, HEADS], F16, kind="ExternalInput")
    QBN = nc.dram_tensor("qb_null", [1, HEADS], F16, kind="ExternalInput")
    VNEM = nc.dram_tensor("vne_m", [16, HEADS * (HD + 1)], F16,
                          kind="ExternalInput")
    VNOM = nc.dram_tensor("vno_m", [16, HEADS * 128], F16,
                          kind="ExternalInput")
    KQSC = nc.dram_tensor("kqsc", [HD], F32, kind="ExternalInput")
    LOG = nc.dram_tensor("ln_out_g", [DIM], F32, kind="ExternalInput")
    LOB = nc.dram_tensor("ln_out_b", [DIM], F32, kind="ExternalInput")
    OUT = nc.dram_tensor("out", [ROWS, DIM], F32, kind="ExternalOutput")

    with tile.TileContext(nc) as tc, ExitStack() as ctx:
        consts = ctx.enter_context(tc.tile_pool(name="consts", bufs=1))
        weights = ctx.enter_context(tc.tile_pool(name="weights", bufs=1))
        kvpool = ctx.enter_context(tc.tile_pool(name="kvpool", bufs=1))
        small = ctx.enter_context(tc.tile_pool(name="small", bufs=3))
        sq_pool = ctx.enter_context(tc.tile_pool(name="sqp", bufs=2))

        # PSUM: psA(3: pq/scores/pf) + mi(1: transposes) +
        #       po(4: pq_null/en_ptr/po_e/po_o/pbr) = 8 banks
        psA = ctx.enter_context(tc.tile_pool(name="psA", bufs=3, space="PSUM"))
        ps_mi = ctx.enter_context(tc.tile_pool(name="ps_mi", bufs=1, space="PSUM"))
        ps_po = ctx.enter_context(tc.tile_pool(name="ps_po", bufs=4, space="PSUM"))

        # ---------------- input DMAs, priority order ----------------
        ctx_t = []
        for i in range(2):
            ct = kvpool.tile([128, DIM], F32, tag=f"ctx{i}", name=f"ctx{i}")
            nc.sync.dma_start(out=ct[:], in_=CTX[i * 128:(i + 1) * 128, :])
            ctx_t.append(ct)
        xin = ctx.enter_context(tc.tile_pool(name="xin", bufs=6))
        xnp = ctx.enter_context(tc.tile_pool(name="xnp", bufs=8))
        x0_tiles = []
        for s in range(NSUB):
            x_t = xin.tile([128, DIM], F32, tag="x")
            nc.sync.dma_start(out=x_t[:], in_=XS[s * 128:(s + 1) * 128, :])
            x0_tiles.append(x_t)
        wkv_ctx = ExitStack()
        wkvp = wkv_ctx.enter_context(tc.tile_pool(name="wkvp", bufs=1))
        wkv_sb = wkvp.tile([128, 8, 2 * DIM], F16)
        for kc in range(8):
            nc.sync.dma_start(out=wkv_sb[:, kc, :],
                              in_=WKV[kc * 128:(kc + 1) * 128, :])
        kqsc = consts.tile([128, HD], F32)
        _load_bcast(nc, kqsc, KQSC[:])
        kvbias = kvpool.tile([1, 2 * DIM], F16)
        nc.sync.dma_start(out=kvbias[0:1, :], in_=KVBIAS[:, :])
        wq_sb = weights.tile([128, 8, DIM], F16)
        for kc in range(8):
            nc.sync.dma_start(out=wq_sb[:, kc, :], in_=WQ[kc * 128:(kc + 1) * 128, :])
        qbias = kvpool.tile([1, DIM], F16)
        nc.sync.dma_start(out=qbias[0:1, :], in_=QBIAS[:, :])
        wq_null = weights.tile([128, 8, HEADS], F16)
        nc.sync.dma_start(
            out=wq_null[:],
            in_=bass.AP(tensor=WQN[0:DIM, :].tensor, offset=0,
                        ap=[[HEADS, 128], [128 * HEADS, 8], [1, HEADS]]))
        qb_null = kvpool.tile([1, HEADS], F16)
        nc.sync.dma_start(out=qb_null[0:1, :], in_=QBN[:, :])
        vne_m = kvpool.tile([16, HEADS, HD + 1], F16)
        nc.sync.dma_start(out=vne_m[:].rearrange("p h d -> p (h d)"),
                          in_=VNEM[:, :])
        vno_m = kvpool.tile([16, HEADS, 128], F16)
        nc.sync.dma_start(out=vno_m[:].rearrange("p h d -> p (h d)"),
                          in_=VNOM[:, :])
        log32 = consts.tile([128, DIM], F32)
        _load_bcast(nc, log32, LOG[:])
        lob = consts.tile([128, DIM], F32)
        _load_bcast(nc, lob, LOB[:])
        wo_sb = weights.tile([128, 8, DIM], F16)
        for kc in range(8):
            nc.sync.dma_start(out=wo_sb[:, kc, :], in_=WO[kc * 128:(kc + 1) * 128, :])

        # ---------------- small consts ----------------
        ident = consts.tile([128, 128], F16)
        make_identity(nc, ident)
        # one act table for the whole kernel (Exp/Ln/Identity/Square/Copy all
        # live in natural_log_exp_and_others) - pre-loading it stops
        # insert_act_table_loads from thrashing between per-func defaults
        from concourse.hw_specs import get_activation_tables
        _tabs = get_activation_tables(nc.m.arch)
        _nle_idx = list(_tabs).index("natural_log_exp_and_others")
        nc.scalar.add_instruction(mybir.InstLoadActFuncSet(
            name=f"I-{nc.next_id()}", ins=[], outs=[],
            act_func_set_id=_nle_idx))
        eps_tile = consts.tile([128, 1], F32)
        nc.vector.memset(eps_tile[:], LN_EPS)
        tiny_tile = consts.tile([128, 1], F32)
        nc.vector.memset(tiny_tile[:], 1e-12)
        onesf = consts.tile([128, 1], F32)
        nc.vector.memset(onesf[:], 1.0)
        expb = consts.tile([128, 1], F32)
        nc.vector.memset(expb[:], EXPB)
        ones64 = consts.tile([128, HD], F16)
        nc.vector.memset(ones64[:], 1.0)
        ones_row = consts.tile([1, 128], F16)
        nc.vector.tensor_copy(ones_row[0:1, :], onesf[0:1, 0:1].broadcast_to([1, 128]))
        log16 = consts.tile([128, DIM], F16)
        nc.vector.tensor_copy(log16[:], log32[:])

        # ---------------- phase K: context -> kT, v_e, v_o ----------------
        kT = kvpool.tile([128, 8, T], F16)
        v_e = kvpool.tile([128, 2, 8, HD + 1], F16)
        nc.vector.tensor_copy(
            v_e[:, :, :, HD:HD + 1],
            onesf[:, 0:1].unsqueeze(1).unsqueeze(1).broadcast_to([128, 2, 8, 1]))
        v_o = kvpool.tile([128, 2, 8, 128], F16)
        nc.vector.memset(v_o[:, :, :, 0:64], 0.0)
        nc.vector.tensor_copy(
            v_o[:, :, :, 0:1],
            onesf[:, 0:1].unsqueeze(1).unsqueeze(1).broadcast_to([128, 2, 8, 1]))

        with ExitStack() as kctx:
            pk = kctx.enter_context(tc.tile_pool(name="pk", bufs=2))
            pk1 = kctx.enter_context(tc.tile_pool(name="pk1", bufs=1))

            cnT = pk1.tile([128, 8, T], F16)
            for i in range(2):
                rstd, negmr = _emit_ln_stats(
                    nc, small, [ctx_t[i][:, 0:512], ctx_t[i][:, 512:1024]],
                    eps_tile)
                cn = pk.tile([128, DIM], F16, tag="cn")
                nc.scalar.activation(out=cn[:], in_=ctx_t[i][:], func=AF.Identity,
                                     bias=negmr[:], scale=rstd[:])
                ptr = ps_mi.tile([128, 8, 128], F16, tag="mi")
                for t in range(8):
                    nc.tensor.transpose(ptr[:, t, :],
                                        cn[:, t * 128:(t + 1) * 128],
                                        ident[:])
                nc.vector.tensor_copy(cnT[:, :, i * 128:(i + 1) * 128], ptr[:])

            for i in range(2):
                for which in (0, 1):  # 0 = k, 1 = v
                    ph = [psA.tile([128, 512], F32, tag="ph", name=f"ph{_i}")
                          for _i in range(2)]
                    for half in range(2):
                        col0 = which * DIM + half * 512
                        nc.tensor.matmul(ph[half][:], lhsT=ones_row[0:1, :],
                                         rhs=kvbias[0:1, col0:col0 + 512],
                                         start=True, stop=False)
                        for kc in range(8):
                            nc.tensor.matmul(
                                ph[half][:],
                                lhsT=cnT[:, kc, i * 128:(i + 1) * 128],
                                rhs=wkv_sb[:, kc, col0:col0 + 512],
                                start=False, stop=(kc == 7))
                    if which == 0:
                        kfin = pk.tile([128, DIM], F16, tag="kfin")
                        _emit_l2norm_heads(
                            nc, small, sq_pool,
                            kfin[:].rearrange("p (h d) -> p h d", d=HD),
                            [ph[0][:], ph[1][:]], kqsc, tiny_tile)
                        ptr = ps_mi.tile([128, 8, 128], F16, tag="mi")
                        for t in range(8):
                            nc.tensor.transpose(
                                ptr[:, t, :],
                                kfin[:, t * 128:(t + 1) * 128],
                                ident[:])
                        nc.vector.tensor_copy(
                            kT[:, :, i * 128:(i + 1) * 128], ptr[:])
                    else:
                        for half in range(2):
                            ph3 = ph[half][:].rearrange("p (h d) -> p h d", d=HD)
                            for hh in range(8):
                                h = half * 8 + hh
                                c = h // 2
                                if h % 2 == 0:
                                    nc.vector.tensor_copy(
                                        v_e[:, i, c, 0:HD], ph3[:, hh, :])
                                else:
                                    nc.vector.tensor_copy(
                                        v_o[:, i, c, 64:128], ph3[:, hh, :])
        wkv_ctx.close()

        # ---------------- main loop pools ----------------
        xnTp = ctx.enter_context(tc.tile_pool(name="xnTp", bufs=2))
        qfp = ctx.enter_context(tc.tile_pool(name="qfp", bufs=2))
        qTp = ctx.enter_context(tc.tile_pool(name="qTp", bufs=2))
        etp = ctx.enter_context(tc.tile_pool(name="etp", bufs=4))
        rp = ctx.enter_context(tc.tile_pool(name="rp", bufs=2))
        rbcp = ctx.enter_context(tc.tile_pool(name="rbcp", bufs=2))
        enp = ctx.enter_context(tc.tile_pool(name="enp", bufs=2))
        nsp = ctx.enter_context(tc.tile_pool(name="nsp", bufs=4))
        outTp = ctx.enter_context(tc.tile_pool(name="outTp", bufs=2))
        obp = ctx.enter_context(tc.tile_pool(name="obp", bufs=2))
        obgp = ctx.enter_context(tc.tile_pool(name="obgp", bufs=2))
        obfp = ctx.enter_context(tc.tile_pool(name="obfp", bufs=2))

        def issue_x_ln(m, x_tiles=None):
            xns = []
            for s in range(NSUB):
                r0 = m * QMACRO + s * 128
                if x_tiles is not None:
                    x_t = x_tiles[s]
                else:
                    x_t = xin.tile([128, DIM], F32, tag="x")
                    nc.sync.dma_start(out=x_t[:], in_=XS[r0:r0 + 128, :])
                rstd, negmr = _emit_ln_stats(
                    nc, small, [x_t[:, 0:512], x_t[:, 512:1024]], eps_tile)
                xn = xnp.tile([128, DIM], F16, tag="xn")
                nc.scalar.activation(out=xn[:], in_=x_t[:], func=AF.Identity,
                                     bias=negmr[:], scale=rstd[:])
                xns.append(xn)
            return xns

        def emit_B(m, xns):
            """Transposes + q projection + l2norm + null cols for macro m.
            The qf transposes lag one subtile behind the projection matmuls
            so PE's in-order queue never waits on the l2norm chain."""
            qT = qTp.tile([128, 8, QMACRO], F16, tag="qT")
            en_all = enp.tile([HEADS, QMACRO], F16, tag="en")
            pend = []

            def finish(s, qf, null_s):
                ptr2 = ps_mi.tile([128, 8, 128], F16, tag="mi")
                for t in range(8):
                    nc.tensor.transpose(ptr2[:, t, :],
                                        qf[:, t * 128:(t + 1) * 128],
                                        ident[:])
                nc.vector.tensor_copy(qT[:, :, s * 128:(s + 1) * 128], ptr2[:])
                en_ptr = ps_po.tile([HEADS, 128], F16, tag="po")
                nc.tensor.transpose(en_ptr[:], null_s[:], ident[:])
                nc.scalar.activation(out=en_all[:, s * 128:(s + 1) * 128],
                                     in_=en_ptr[:], func=AF.Exp,
                                     bias=expb[0:HEADS, :], scale=SCALE)

            for s in range(NSUB):
                xn = xns[s]
                xnT = xnTp.tile([128, 8, 128], F16, tag="xnT")
                ptr = ps_mi.tile([128, 8, 128], F16, tag="mi")
                for t in range(8):
                    nc.tensor.transpose(ptr[:, t, :],
                                        xn[:, t * 128:(t + 1) * 128],
                                        ident[:])
                nc.scalar.copy(xnT[:], ptr[:])
                pq = [psA.tile([128, 512], F32, tag="ph", name=f"pq{_i}")
                      for _i in range(2)]
                pq_null = ps_po.tile([128, HEADS], F32, tag="po")
                for half in range(2):
                    nc.tensor.matmul(pq[half][:], lhsT=ones_row[0:1, :],
                                     rhs=qbias[0:1, half * 512:(half + 1) * 512],
                                     start=True, stop=False)
                    for kc in range(8):
                        nc.tensor.matmul(
                            pq[half][:],
                            lhsT=xnT[:, kc, :],
                            rhs=wq_sb[:, kc, half * 512:(half + 1) * 512],
                            start=False, stop=(kc == 7))
                nc.tensor.matmul(pq_null[:], lhsT=ones_row[0:1, :],
                                 rhs=qb_null[0:1, :], start=True, stop=False)
                for kc in range(8):
                    nc.tensor.matmul(pq_null[:], lhsT=xnT[:, kc, :],
                                     rhs=wq_null[:, kc, :],
                                     start=False, stop=(kc == 7))
                if pend:
                    finish(*pend.pop())
                qf = qfp.tile([128, DIM], F16, tag="qf")
                rn = _emit_l2norm_heads(nc, small, sq_pool,
                                        qf[:].rearrange("p (h d) -> p h d", d=HD),
                                        [pq[0][:], pq[1][:]], None, tiny_tile)
                null_s = nsp.tile([128, HEADS], F16, tag="ns")
                nc.vector.tensor_mul(null_s[:], pq_null[:], rn[:])
                pend.append((s, qf, null_s))
            finish(*pend.pop())
            return qT, en_all

        def emit_head(m, qT, en_all):
            outT = outTp.tile([128, 8, QMACRO], F16, tag="outT")
            for c in range(8):
                h0, h1 = 2 * c, 2 * c + 1
                for par in range(2):  # 0 = even/lower head, 1 = odd/upper
                    jb = par * HD
                    kT_h = kT[jb:jb + HD, c, :]
                    qT_h = qT[jb:jb + HD, c, :]
                    ets = []
                    for kc in range(2):
                        ps_s = psA.tile([128, QMACRO], F32, tag="ph",
                                        name=f"s{kc}")
                        nc.tensor.matmul(ps_s[:],
                                         lhsT=kT_h[:, kc * 128:(kc + 1) * 128],
                                         rhs=qT_h, start=True, stop=True)
                        et = etp.tile([128, QMACRO], F16, tag="et")
                        nc.scalar.activation(out=et[:], in_=ps_s[:],
                                             func=AF.Exp, bias=expb[:],
                                             scale=SCALE)
                        ets.append(et)
                    if par == 0:
                        po = ps_po.tile([HD + 1, QMACRO], F32, tag="po")
                        nc.tensor.matmul(po[:], lhsT=v_e[:, 0, c, :],
                                         rhs=ets[0][:], start=True, stop=False)
                        nc.tensor.matmul(po[:], lhsT=v_e[:, 1, c, :],
                                         rhs=ets[1][:], start=False, stop=False)
                        nc.tensor.matmul(po[:], lhsT=vne_m[:, h0, :],
                                         rhs=en_all[:, :],
                                         start=False, stop=True)
                        r16 = rp.tile([HD + 1, QMACRO], F16, tag="r16")
                        with nc.allow_low_precision(reason="1/sum f16 5e-4"):
                            nc.vector.reciprocal(r16[HD:HD + 1, :],
                                                 po[HD:HD + 1, :])
                        pbr = ps_po.tile([HD, QMACRO], F32, tag="po")
                        nc.tensor.matmul(pbr[:], lhsT=ones64[HD:HD + 1, :],
                                         rhs=r16[HD:HD + 1, :],
                                         start=True, stop=True)
                        rbf = rbcp.tile([HD, QMACRO], F32, tag="rbf")
                        if c % 2 == 0:
                            nc.scalar.copy(rbf[:], pbr[:])
                        else:
                            nc.vector.tensor_copy(rbf[:], pbr[:])
                        nc.vector.tensor_mul(outT[0:HD, c, :], po[0:HD, :],
                                             rbf[:])
                    else:
                        po = ps_po.tile([128, QMACRO], F32, tag="po")
                        nc.tensor.matmul(po[:], lhsT=v_o[:, 0, c, :],
                                         rhs=ets[0][:], start=True, stop=False)
                        nc.tensor.matmul(po[:], lhsT=v_o[:, 1, c, :],
                                         rhs=ets[1][:], start=False, stop=False)
                        nc.tensor.matmul(po[:], lhsT=vno_m[:, h1, :],
                                         rhs=en_all[:, :],
                                         start=False, stop=True)
                        r_t = rp.tile([1, QMACRO], F32, tag="r")
                        nc.vector.reciprocal(r_t[0:1, :], po[0:1, :])
                        rbc = rbcp.tile([128, QMACRO], F32, tag="rbc")
                        nc.gpsimd.partition_broadcast(rbc[:, :], r_t[0:1, :])
                        nc.vector.tensor_mul(outT[64:128, c, :], po[64:128, :],
                                             rbc[64:128, :])
            return outT

        def emit_out(m, outT):
            for s in range(NSUB):
                r0 = m * QMACRO + s * 128
                pf = [psA.tile([128, 512], F32, tag="ph", name=f"pf{_i}")
                      for _i in range(2)]
                for half in range(2):
                    for kc in range(8):
                        nc.tensor.matmul(
                            pf[half][:],
                            lhsT=outT[:, kc, s * 128:(s + 1) * 128],
                            rhs=wo_sb[:, kc, half * 512:(half + 1) * 512],
                            start=(kc == 0), stop=(kc == 7))
                rstd, negmr = _emit_ln_stats(nc, small, [pf[0][:], pf[1][:]],
                                             eps_tile)
                ob16 = obp.tile([128, DIM], F16, tag="ob16")
                for half in range(2):
                    nc.scalar.activation(out=ob16[:, half * 512:(half + 1) * 512],
                                         in_=pf[half][:], func=AF.Identity,
                                         bias=negmr[:], scale=rstd[:])
                obg = obgp.tile([128, DIM], F16, tag="obg")
                nc.vector.tensor_mul(obg[:], ob16[:], log16[:])
                obf = obfp.tile([128, DIM], F32, tag="obf")
                nc.gpsimd.tensor_add(obf[:], obg[:], lob[:])
                nc.sync.dma_start(out=OUT[r0:r0 + 128, :], in_=obf[:])

        xns = issue_x_ln(0, x0_tiles)
        qT_cur, en_cur = emit_B(0, xns)
        for m in range(NMACRO):
            if m + 1 < NMACRO:
                xns = issue_x_ln(m + 1)
            outT = emit_head(m, qT_cur, en_cur)
            if m + 1 < NMACRO:
                qT_cur, en_cur = emit_B(m + 1, xns)
            emit_out(m, outT)

    nc.compile()
    return nc


_NC_CACHE = None


def _host_fold(inputs):
    """Fold LN gains/biases into the projection weights and precompute the
    null-kv artifacts (all f32 on host, cast to f16 at the end)."""
    Wq = np.asarray(inputs["Wq"], np.float32)
    Wkv = np.asarray(inputs["Wkv"], np.float32)
    Wo = np.asarray(inputs["Wo"], np.float32)
    nkv = np.asarray(inputs["null_kv"], np.float32)
    qs = np.asarray(inputs["q_scale"], np.float32)
    ks = np.asarray(inputs["k_scale"], np.float32)
    lig = np.asarray(inputs["ln_in_g"], np.float32)
    lib = np.asarray(inputs["ln_in_b"], np.float32)
    lcg = np.asarray(inputs["ln_ctx_g"], np.float32)
    lcb = np.asarray(inputs["ln_ctx_b"], np.float32)

    Wq_s = lig[:, None] * Wq
    qbias = (lib @ Wq)[None, :]
    Wkv_s = lcg[:, None] * Wkv
    kvbias = (lcb @ Wkv)[None, :]

    kn = nkv[0]
    w = kn / max(float(np.sqrt((kn * kn).sum())), NORM_EPS) * qs * ks  # [64]
    wq_null = np.einsum("dhe,e->dh",
                        Wq_s.reshape(DIM, HEADS, HD), w)  # [1024, 16]
    qb_null = (qbias.reshape(HEADS, HD) @ w)[None, :]  # [1, 16]

    vn = nkv[1]
    vne = np.zeros((16, HEADS, HD + 1), np.float32)
    vno = np.zeros((16, HEADS, 128), np.float32)
    for h in range(HEADS):
        vne[h, h, 0:HD] = vn
        vne[h, h, HD] = 1.0
        vno[h, h, 0] = 1.0
        vno[h, h, 64:128] = vn

    return {
        "Wq": Wq_s.astype(np.float16),
        "Wkv": Wkv_s.astype(np.float16),
        "Wo": Wo.astype(np.float16),
        "qbias": qbias.astype(np.float16),
        "kvbias": kvbias.astype(np.float16),
        "wq_null": wq_null.astype(np.float16),
        "qb_null": qb_null.astype(np.float16),
        "vne_m": vne.reshape(16, HEADS * (HD + 1)).astype(np.float16),
        "vno_m": vno.reshape(16, HEADS * 128).astype(np.float16),
        "kqsc": (qs * ks).astype(np.float32),
        "ln_out_g": np.asarray(inputs["ln_out_g"], np.float32),
        "ln_out_b": np.asarray(inputs["ln_out_b"], np.float32),
    }


def kernel(**inputs):
    global _NC_CACHE
    if _NC_CACHE is None:
        _NC_CACHE = build_nc()
    nc = _NC_CACHE

    x = np.asarray(inputs["x"], np.float32)
    context = np.asarray(inputs["context"], np.float32)
    shared = _host_fold(inputs)
    B, N, _ = x.shape
    in_maps = []
    for c in range(N_CORES):
        b, n0 = c // 2, (c % 2) * ROWS
        in_maps.append({"xs": np.ascontiguousarray(x[b, n0:n0 + ROWS]),
                        "ctx": np.ascontiguousarray(context[b]), **shared})

    res = run_bass_kernel_spmd(nc, in_maps, list(range(N_CORES)))

    out = np.empty((B, N, DIM), np.float32)
    for c in range(N_CORES):
        b, n0 = c // 2, (c % 2) * ROWS
        out[b, n0:n0 + ROWS] = res.results[c]["out"]
    return out
